# revision 1
# baseline (speedup 1.0000x reference)
"""Trainium2 Bass kernel for multi-head global attention (the
"DeformableAttention" module whose relative-position-bias path is inactive).

Reference computation (per batch b):
    qkv = x @ w_qkv.T + b_qkv            # [N, 3C]
    q, k, v = split/reshape to [nh, N, hd]
    attn = softmax((q @ k.T) * hd**-0.5)
    out  = (attn @ v) merged heads       # [N, C]
    y    = out @ w_proj.T + b_proj

Sharding: data-parallel over batch B=16 across 8 NeuronCores (2 batches/core).
No collectives.

Device-side design (per core, per batch):
  * x is staged pre-transposed (xT, [C, tokens]) so every matmul contraction
    dim lands on SBUF partitions without any on-device transpose.
  * Q^T, K^T ([hd, N]) are produced per-head straight from the QKV projection
    (head-sized M=96 stationary tiles); V in natural [N, nh*(hd+1)] layout
    with an interleaved ones-column per head (built by a rank-1 bias matmul).
  * Scores are computed transposed (S^T[k, q] blocks), softmax's exp runs on
    ScalarE with the 1/sqrt(hd) scale fused, and the row-sums fall out of the
    P~ @ [V | 1] matmul for free (row hd of the PSUM output).
  * O^T is copied out of PSUM immediately (frees the accumulator), normalized
    by the broadcast reciprocal row-sum (partition-broadcast via a DRAM
    bounce + stride-0 DMA), and DMA-repacked into a dense [C, N] attn^T
    buffer (partition-shifting SBUF->SBUF DMA).
  * Output projection contracts attn^T against w_proj.T in 6 dense 128-chunks,
    producing final [token, C] tiles in natural layout for direct DMA out.

All matmul operands are float32r (fp32 bit layout; the PE processes it at
bf16 rate for free dims >= 256, ~4x faster than strict fp32). Set
KERNEL_MM_DT=f32 for exact-fp32 matmuls instead.
"""

import os
import sys

sys.path.insert(0, "/opt/trn_rl_repo")

# The Bass->PJRT execution path needs jax to discover the axon-tunneled
# NeuronCores; a stray JAX_PLATFORMS=cpu (e.g. set for a jax reference run)
# would hide them. Only effective if jax hasn't been imported yet.
if "jax" not in sys.modules and "axon" not in os.environ.get("JAX_PLATFORMS", "axon"):
    os.environ.pop("JAX_PLATFORMS", None)

import numpy as np

import concourse.bass as bass
import concourse.mybir as mybir
import concourse.tile as tile
from concourse import bacc
from concourse.bass_utils import run_bass_kernel_spmd

# Problem constants (hardcoded per the task contract).
B, N, C = 16, 1024, 768
NH, HD = 8, 96
NCORES = 8
BPC = B // NCORES  # batches per core = 2
CC = C // 128  # contraction chunks of 128 = 6
KC = N // 128  # key chunks per batch = 8
QH = N // 512  # query halves = 2
TOKC = N // 128  # token chunks for V projection = 8
QC = N // 128  # query chunks for output projection = 8
HDA = HD + 1  # head dim + ones column = 97
VW = NH * HDA  # augmented V width = 776
SCALE = float(HD) ** -0.5

F32 = mybir.dt.float32

_BUILD_CACHE = {}


def _mm_dt():
    return F32 if os.environ.get("KERNEL_MM_DT") == "f32" else mybir.dt.float32r


def _build(qk_bias: bool, p_bias: bool):
    """Build + compile the single-core Bass program (shared SPMD across cores)."""
    knobs = tuple(
        int(os.environ.get(k, d))
        for k, d in (
            ("PT_BUFS", 3),
            ("QKT_BUFS", 2),
            ("WQKH_BUFS", 2),
            ("SP_BUFS", 2),
            ("OP_BUFS", 2),
            ("MP_BUFS", 2),
            ("RB_BUFS", 1),
            ("OUT_BUFS", 2),
            ("OT_BUFS", 2),
            ("ON_BUFS", 2),
        )
    )
    key = (qk_bias, p_bias, os.environ.get("KERNEL_MM_DT", "f32r"), knobs)
    if key in _BUILD_CACHE:
        return _BUILD_CACHE[key]
    ptb, qktb, wqkhb, spb, opb, mpb, rbb, outb, otb, onb = knobs

    mmdt = _mm_dt()

    nc = bacc.Bacc("TRN2", target_bir_lowering=False, debug=False)

    xT_d = nc.dram_tensor("xT", [C, BPC * N], mmdt, kind="ExternalInput")
    wqk_d = nc.dram_tensor("wqk", [C, 2 * NH * HD], mmdt, kind="ExternalInput")
    wv_d = nc.dram_tensor("wv", [C, VW], mmdt, kind="ExternalInput")
    wp_d = nc.dram_tensor("wp", [C, C], mmdt, kind="ExternalInput")
    bvaug_d = nc.dram_tensor("bvaug", [1, VW], mmdt, kind="ExternalInput")
    ones_d = nc.dram_tensor("ones", [1, 128], mmdt, kind="ExternalInput")
    vones_d = nc.dram_tensor("vones", [128, TOKC, NH], mmdt, kind="ExternalInput")
    if qk_bias:
        bqk_d = nc.dram_tensor("bqk", [HD, 2 * NH], F32, kind="ExternalInput")
    if p_bias:
        bp_d = nc.dram_tensor("bp", [1, C], mmdt, kind="ExternalInput")
    y_d = nc.dram_tensor("y", [BPC, N, C], F32, kind="ExternalOutput")

    xT_re = xT_d.rearrange("(o p) t -> p o t", p=128)
    wqk_re = wqk_d.rearrange("(o p) f -> p o f", p=128)
    wv_re = wv_d.rearrange("(o p) f -> p o f", p=128)
    wp_re = wp_d.rearrange("(o p) f -> p o f", p=128)

    EXP = mybir.ActivationFunctionType.Exp

    with tile.TileContext(nc) as tc:
        with (
            tc.tile_pool(name="wpool", bufs=1) as wpool,
            tc.tile_pool(name="wqkh_pool", bufs=wqkhb) as wqkh_pool,
            tc.tile_pool(name="xpool", bufs=1) as xpool,
            tc.tile_pool(name="qkt_pool", bufs=qktb) as qkt_pool,
            tc.tile_pool(name="vpool", bufs=1) as vpool,
            tc.tile_pool(name="pt_pool", bufs=ptb) as pt_pool,
            tc.tile_pool(name="attn_pool", bufs=2) as attn_pool,
            tc.tile_pool(name="rb_pool", bufs=rbb) as rb_pool,
            tc.tile_pool(name="ot_pool", bufs=otb) as ot_pool,
            tc.tile_pool(name="on_pool", bufs=onb) as on_pool,
            tc.tile_pool(name="rdram_pool", bufs=2, space="DRAM") as rdram_pool,
            tc.tile_pool(name="out_pool", bufs=outb) as out_pool,
            tc.tile_pool(name="spsum", bufs=spb, space="PSUM") as spsum,
            tc.tile_pool(name="opsum_pool", bufs=opb, space="PSUM") as opsum_pool,
            tc.tile_pool(name="mpsum", bufs=mpb, space="PSUM") as mpsum,
        ):
            # --- resident weights/constants ---
            wv_sb = wpool.tile([128, CC, VW], mmdt, tag="wv")
            nc.scalar.dma_start(wv_sb[:, 0:3, 0:512], wv_re[:, 0:3, 0:512])
            nc.scalar.dma_start(wv_sb[:, 3:CC, 0:512], wv_re[:, 3:CC, 0:512])
            nc.scalar.dma_start(wv_sb[:, :, 512:VW], wv_re[:, :, 512:VW])
            # wp is not needed until the first output projection (~100us in);
            # issue its load after the first batch's V projection to keep the
            # startup-critical DMAs (x, wv, wqk head 0) ahead of it.
            wp_sb = wpool.tile([128, CC, C], mmdt, tag="wp")
            bvaug_sb = wpool.tile([1, VW], mmdt, tag="bvaug")
            nc.scalar.dma_start(bvaug_sb[:], bvaug_d[:])
            ones_sb = wpool.tile([1, 128], mmdt, tag="ones")
            nc.scalar.dma_start(ones_sb[:], ones_d[:])
            if qk_bias:
                bqk_sb = wpool.tile([HD, 2 * NH], F32, tag="bqk")
                nc.scalar.dma_start(bqk_sb[:], bqk_d[:])
            if p_bias:
                bp_sb = wpool.tile([1, C], mmdt, tag="bp")
                nc.scalar.dma_start(bp_sb[:], bp_d[:])

            def emit_vproj(b):
                """Stage batch b's x^T and project V (ones-augmented)."""
                xTb = xpool.tile([128, CC, N], mmdt, tag="xTb", name="xTb")
                for xh in range(4):
                    nc.sync.dma_start(
                        xTb[:, :, xh * (N // 4) : (xh + 1) * (N // 4)],
                        xT_re[:, :, b * N + xh * (N // 4) : b * N + (xh + 1) * (N // 4)],
                    )
                v_sb = vpool.tile([128, TOKC, VW], mmdt, tag="v", name="v_sb")
                v_bias = bool(qk_bias)  # b_qkv nonzero => v bias nonzero path
                for t in range(TOKC):
                    for lo, hi in ((0, 512), (512, VW)):
                        vps = mpsum.tile([128, 512], F32, tag="mpsum", name="vps")
                        w = hi - lo
                        for cc in range(CC):
                            nc.tensor.matmul(
                                vps[:, :w],
                                xTb[:, cc, t * 128 : (t + 1) * 128],
                                wv_sb[:, cc, lo:hi],
                                start=(cc == 0),
                                stop=(cc == CC - 1 and not v_bias),
                            )
                        if v_bias:
                            # bias + per-head ones-columns via rank-1 update
                            nc.tensor.matmul(
                                vps[:, :w],
                                ones_sb[:, :],
                                bvaug_sb[:, lo:hi],
                                start=False,
                                stop=True,
                            )
                        nc.scalar.activation(v_sb[:, t, lo:hi], vps[:, :w], mybir.ActivationFunctionType.Copy)
                if not v_bias:
                    # fill each head's ones-column with a single strided DMA
                    nc.sync.dma_start(
                        v_sb.rearrange("p t (h a) -> p t h a", a=HDA)[:, :, :, HD],
                        vones_d[:],
                    )
                return xTb, v_sb

            def emit_heads(b, xTb, v_sb):
                # densely packed attn^T [C, N]: head h occupies rows
                # HD*h .. HD*h+HD; every row is written (no junk partitions).
                attnT = attn_pool.tile([128, CC, N], mmdt, tag="attnT", name="attnT")

                def qkproj_steps(h):
                    """Generator: head h's Q^T/K^T projection as 8 emission
                    steps of 3 matmuls each (plus the PSUM->SBUF copy when a
                    (f, tq) accumulation group completes). Interleaving these
                    into the previous head's attention loop keeps the PE fed
                    while ScalarE works through the exps."""
                    wqkh = wqkh_pool.tile([128, CC, 2 * HD], mmdt, tag="wqkh")
                    nc.scalar.dma_start(
                        wqkh[:], wqk_re[:, :, 2 * HD * h : 2 * HD * (h + 1)]
                    )
                    qkt = qkt_pool.tile([128, 2, N], mmdt, tag="qkt")
                    seq = [(f, tq, cc) for f in range(2) for tq in range(QH) for cc in range(CC)]
                    qps = None
                    for step in range(8):
                        for f, tq, cc in seq[3 * step : 3 * step + 3]:
                            if cc == 0:
                                qps = mpsum.tile(
                                    [128, 512], F32, tag="mpsum", name="qps"
                                )
                            nc.tensor.matmul(
                                qps[:HD, :],
                                wqkh[:, cc, HD * f : HD * (f + 1)],
                                xTb[:, cc, tq * 512 : (tq + 1) * 512],
                                start=(cc == 0),
                                stop=(cc == CC - 1),
                            )
                            if cc == CC - 1:
                                dst = qkt[:HD, f, tq * 512 : (tq + 1) * 512]
                                if qk_bias:
                                    nc.scalar.activation(
                                        dst,
                                        qps[:HD, :],
                                        mybir.ActivationFunctionType.Copy,
                                        bias=bqk_sb[:, 2 * h + f : 2 * h + f + 1],
                                    )
                                else:
                                    nc.vector.tensor_copy(dst, qps[:HD, :])
                        yield qkt

                # head 0's projection runs unoverlapped; head h+1's is spread
                # across head h's attention inner loop.
                qkt_cur = None
                for qkt_cur in qkproj_steps(0):
                    pass

                for h in range(NH):
                    nxt_gen = qkproj_steps(h + 1) if h + 1 < NH else None
                    qkt_next = None

                    # --- attention: S^T blocks, exp, P~ @ [V | 1] ---
                    ops = [
                        opsum_pool.tile([128, 512], F32, tag="opsum", name="ops")
                        for _ in range(QH)
                    ]
                    for kc in range(KC):
                        st = spsum.tile([128, QH, 512], F32, tag="spsum", name="st")
                        for qh in range(QH):
                            nc.tensor.matmul(
                                st[:, qh, :],
                                qkt_cur[:HD, 1, kc * 128 : (kc + 1) * 128],
                                qkt_cur[:HD, 0, qh * 512 : (qh + 1) * 512],
                                start=True,
                                stop=True,
                            )
                        pt = pt_pool.tile([128, QH, 512], mmdt, tag="pt", name="pt")
                        nc.scalar.activation(pt[:], st[:], EXP, scale=SCALE)
                        for qh in range(QH):
                            nc.tensor.matmul(
                                ops[qh][:HDA, :],
                                v_sb[:, kc, HDA * h : HDA * (h + 1)],
                                pt[:, qh, :],
                                start=(kc == 0),
                                stop=(kc == KC - 1),
                            )
                        if nxt_gen is not None:
                            qkt_next = next(nxt_gen)

                    # --- normalize O^T by 1/rowsum and repack into attn^T ---
                    rb = rb_pool.tile([128, N], F32, tag="rb")
                    rd = rdram_pool.tile([1, N], F32, tag="rd", name="rd")
                    for qh in range(QH):
                        qs = slice(qh * 512, (qh + 1) * 512)
                        # free the PSUM accumulator ASAP
                        ot = ot_pool.tile([128, 512], F32, tag="ot", name="ot")
                        # reciprocal first: it heads the (DRAM-bounce) broadcast
                        # chain; the O^T copy overlaps with the bounce DMAs
                        nc.vector.reciprocal(rb[HD : HD + 1, qs], ops[qh][HD : HD + 1, :])
                        nc.vector.tensor_copy(ot[:HD, :], ops[qh][:HD, :])
                        nc.gpsimd.dma_start(rd[:, qs], rb[HD : HD + 1, qs])
                        src = rd[:, qs]
                        bsrc = bass.AP(
                            tensor=src.tensor,
                            offset=src.offset,
                            ap=[[0, HD]] + [list(x) for x in src.ap[1:]],
                        )
                        nc.gpsimd.dma_start(rb[:HD, qs], bsrc)
                        oN = on_pool.tile([128, 512], mmdt, tag="oN", name="oN")
                        nc.vector.tensor_tensor(
                            oN[:HD, :],
                            ot[:HD, :],
                            rb[:HD, qs],
                            mybir.AluOpType.mult,
                        )
                        # repack rows HD*h..HD*h+HD of attn^T (may straddle one
                        # 128-partition chunk boundary -> up to two DMAs)
                        r0 = HD * h
                        cc0, p0 = divmod(r0, 128)
                        len1 = min(HD, 128 - p0)
                        nc.gpsimd.dma_start(
                            attnT[p0 : p0 + len1, cc0, qs], oN[:len1, :]
                        )
                        if len1 < HD:
                            nc.gpsimd.dma_start(
                                attnT[0 : HD - len1, cc0 + 1, qs],
                                oN[len1:HD, :],
                            )

                    if qkt_next is not None:
                        qkt_cur = qkt_next

                return attnT

            def emit_proj(b, attnT, qcs=None):
                """Output projection: dense 6-chunk contraction."""
                for qc in qcs if qcs is not None else range(QC):
                    out_sb = out_pool.tile([128, C], F32, tag="out", name="out_sb")
                    for n in range(2):
                        pps = mpsum.tile([128, 512], F32, tag="mpsum", name="pps")
                        ns = slice(n * 384, (n + 1) * 384)
                        for cc in range(CC):
                            nc.tensor.matmul(
                                pps[:, :384],
                                attnT[:, cc, qc * 128 : (qc + 1) * 128],
                                wp_sb[:, cc, ns],
                                start=(cc == 0),
                                stop=(cc == CC - 1 and not p_bias),
                            )
                        if p_bias:
                            nc.tensor.matmul(
                                pps[:, :384],
                                ones_sb[:, :],
                                bp_sb[:, ns],
                                start=False,
                                stop=True,
                            )
                        nc.scalar.activation(out_sb[:, ns], pps[:, :384], mybir.ActivationFunctionType.Copy)
                    nc.sync.dma_start(y_d[b, qc * 128 : (qc + 1) * 128, :], out_sb[:])

            # Emission (≈ static engine) order V0 H0 V1 P0 H1 P1: batch 1's V
            # projection fills the PE while batch 0's normalize tail drains,
            # and proj(0) fills it while batch 1's first heads project.
            xTb0, v0 = emit_vproj(0)
            at0 = emit_heads(0, xTb0, v0)
            xTb1, v1 = emit_vproj(1)
            # wp is first used by proj(0), ~20us after this point lands
            nc.scalar.dma_start(wp_sb[:], wp_re[:])
            at1 = emit_heads(1, xTb1, v1)
            # proj(0) is deferred past heads(1): it fills the PE while batch
            # 1's last normalize chain drains, and proj(1) backfills proj(0)'s
            # own epilogue.
            emit_proj(0, at0)
            emit_proj(1, at1)

    nc.compile()
    _BUILD_CACHE[key] = nc
    return nc


def _prep_shared(w_qkv, b_qkv, w_proj, b_proj):
    """Host-side weight rearrangement shared by all cores."""
    w_qkv = np.ascontiguousarray(w_qkv, dtype=np.float32)
    w_proj = np.ascontiguousarray(w_proj, dtype=np.float32)
    b_qkv = np.asarray(b_qkv, dtype=np.float32)
    b_proj = np.asarray(b_proj, dtype=np.float32)

    # wqk: [C, 2*NH*HD] with column 2*HD*h + HD*f + j = w_qkv row C*f + HD*h + j
    wqk = w_qkv[: 2 * C].reshape(2, NH, HD, C)  # [f, h, j, c]
    wqk_arr = np.ascontiguousarray(
        np.transpose(wqk, (3, 1, 0, 2)).reshape(C, 2 * NH * HD)
    )

    # wv: [C, NH*(HD+1)] with a zero ones-column slot per head
    wv = w_qkv[2 * C :].reshape(NH, HD, C)  # [h, j, c]
    wv_aug = np.zeros((C, NH, HDA), dtype=np.float32)
    wv_aug[:, :, :HD] = np.transpose(wv, (2, 0, 1))
    wv_aug = np.ascontiguousarray(wv_aug.reshape(C, VW))

    # wp: plain transpose [c_in, c_out]
    wp_t = np.ascontiguousarray(w_proj.T)

    # bvaug: v-bias interleaved with 1.0 at each head's ones-column
    bvaug = np.zeros((1, NH, HDA), dtype=np.float32)
    bvaug[0, :, :HD] = b_qkv[2 * C :].reshape(NH, HD)
    bvaug[0, :, HD] = 1.0
    bvaug = bvaug.reshape(1, VW)

    ones = np.ones((1, 128), dtype=np.float32)
    vones = np.ones((128, TOKC, NH), dtype=np.float32)

    qk_bias = bool(np.any(b_qkv[: 2 * C] != 0.0))
    p_bias = bool(np.any(b_proj != 0.0))
    extra = {}
    if qk_bias:
        # [HD, 2*NH] col 2h+f = bias of (f, h)
        bqk = b_qkv[: 2 * C].reshape(2, NH, HD)  # [f, h, j]
        extra["bqk"] = np.ascontiguousarray(
            np.transpose(bqk, (2, 1, 0)).reshape(HD, 2 * NH)
        )
    if p_bias:
        extra["bp"] = np.ascontiguousarray(b_proj.reshape(1, C))

    return wqk_arr, wv_aug, wp_t, bvaug, ones, vones, qk_bias, p_bias, extra


def kernel(x, w_qkv, b_qkv, w_proj, b_proj, H=32, W=32):
    x = np.asarray(x, dtype=np.float32)
    assert x.shape == (B, N, C), x.shape
    assert int(H) * int(W) == N

    wqk_arr, wv_aug, wp_t, bvaug, ones, vones, qk_bias, p_bias, extra = _prep_shared(
        w_qkv, b_qkv, w_proj, b_proj
    )
    nc = _build(qk_bias, p_bias)

    in_maps = []
    for c in range(NCORES):
        xc = x[BPC * c : BPC * (c + 1)].reshape(BPC * N, C)
        xT = np.ascontiguousarray(xc.T)  # [C, BPC*N]
        m = {
            "xT": xT,
            "wqk": wqk_arr,
            "wv": wv_aug,
            "wp": wp_t,
            "bvaug": bvaug,
            "ones": ones,
            "vones": vones,
        }
        m.update(extra)
        in_maps.append(m)

    trace = os.environ.get("KERNEL_TRACE") == "1"
    res = run_bass_kernel_spmd(
        nc, in_maps, core_ids=list(range(NCORES)), trace=trace
    )
    if trace:
        kernel.last_results = res
        print("exec_time_ns:", res.exec_time_ns, "mean:", res.mean_exec_time_ns)
        if res.instructions_and_trace:
            print("trace:", res.instructions_and_trace[1])

    out = np.empty((B, N, C), dtype=np.float32)
    for c in range(NCORES):
        out[BPC * c : BPC * (c + 1)] = res.results[c]["y"]
    return out


if __name__ == "__main__":
    rng = np.random.default_rng(0)
    x = rng.standard_normal((B, N, C), dtype=np.float32)
    w_qkv = rng.standard_normal((3 * C, C), dtype=np.float32) / np.sqrt(C)
    b_qkv = np.zeros(3 * C, np.float32)
    w_proj = rng.standard_normal((C, C), dtype=np.float32) / np.sqrt(C)
    b_proj = np.zeros(C, np.float32)
    y = kernel(x, w_qkv, b_qkv, w_proj, b_proj)
    print("out", y.shape, y.dtype, float(np.abs(y).mean()))



# revision 8
# speedup vs baseline: 1.0551x; 1.0551x over previous
"""Trainium2 Bass kernel for multi-head global attention (the
"DeformableAttention" module whose relative-position-bias path is inactive).

Reference computation (per batch b):
    qkv = x @ w_qkv.T + b_qkv            # [N, 3C]
    q, k, v = split/reshape to [nh, N, hd]
    attn = softmax((q @ k.T) * hd**-0.5)
    out  = (attn @ v) merged heads       # [N, C]
    y    = out @ w_proj.T + b_proj

Sharding: data-parallel over batch B=16 across 8 NeuronCores (2 batches/core).
No collectives.

Device-side design (per core, per batch), all SBUF operands bf16 so every
matmul streams at 1 cycle/row and 128-column stationaries get fast weight
load:
  * x is staged pre-transposed (xT, [C, tokens]).
  * Q^T/K^T are produced by a DENSE projection: 12 chunks of M=128 rows in
    (head, q|k, j) order -- no M=96 underutilization. Head tiles that start
    mid-chunk are realigned to partition 0 by small SBUF->SBUF DMAs; the 4
    chunk-aligned tiles are read in place.
  * V in natural [N, nh*(hd+1)] layout with an interleaved ones-column per
    head.
  * Scores are computed transposed (S^T[k, q] blocks), softmax's exp runs on
    ScalarE with the 1/sqrt(hd) scale fused.
  * AV runs QUERY-NATURAL: out[q, hd] tiles with M=128 (full PE height),
    F=97 bf16; the ones-column row-sum lands on the same partition as its
    queries, so the normalize is a per-partition reciprocal + tensor_scalar
    (no cross-partition broadcast, no DRAM bounce).
  * The normalized per-q-tile [128, C] block (heads side by side) is turned
    into the projection's [C, q] layout by one XBAR DMA-transpose per q-tile.
  * Output projection contracts 6 dense 128-chunks, producing [token, C]
    tiles for direct DMA out.
"""

import os
import sys

sys.path.insert(0, "/opt/trn_rl_repo")

# The Bass->PJRT execution path needs jax to discover the axon-tunneled
# NeuronCores; a stray JAX_PLATFORMS=cpu (e.g. set for a jax reference run)
# would hide them. Only effective if jax hasn't been imported yet.
if "jax" not in sys.modules and "axon" not in os.environ.get("JAX_PLATFORMS", "axon"):
    os.environ.pop("JAX_PLATFORMS", None)

import ml_dtypes
import numpy as np

import concourse.bass as bass
import concourse.mybir as mybir
import concourse.tile as tile
from concourse import bacc
from concourse.bass_utils import run_bass_kernel_spmd

# Problem constants (hardcoded per the task contract).
B, N, C = 16, 1024, 768
NH, HD = 8, 96
NCORES = 8
BPC = B // NCORES  # batches per core = 2
CC = C // 128  # contraction chunks of 128 = 6
KC = N // 128  # key chunks per batch = 8
QH = N // 512  # query halves = 2
TOKC = N // 128  # token chunks = 8
QC = N // 128  # query chunks for output projection = 8
HDA = HD + 1  # head dim + ones column = 97
VW = NH * HDA  # augmented V width = 776
NCH = 2 * NH * HD // 128  # dense Q/K projection chunks = 12
SCALE = float(HD) ** -0.5

F32 = mybir.dt.float32
BF16 = mybir.dt.bfloat16
NPBF16 = ml_dtypes.bfloat16

_BUILD_CACHE = {}


def _qk_tile_geom(h, f):
    """Dense-row geometry of head-tile (h, f): rows r0..r0+95 of the
    (h, f, j) row space land in chunk o at partitions p0.., possibly
    spilling len2 rows into chunk o+1."""
    r0 = 2 * HD * h + HD * f
    o, p0 = divmod(r0, 128)
    len1 = min(HD, 128 - p0)
    return o, p0, len1, HD - len1


def _build(qk_bias: bool, p_bias: bool):
    """Build + compile the single-core Bass program (shared SPMD across cores)."""
    key = (qk_bias, p_bias)
    if key in _BUILD_CACHE:
        return _BUILD_CACHE[key]

    nc = bacc.Bacc("TRN2", target_bir_lowering=False, debug=False)

    xT_d = nc.dram_tensor("xT", [C, BPC * N], BF16, kind="ExternalInput")
    wqk_d = nc.dram_tensor("wqk", [C, 2 * NH * HD], BF16, kind="ExternalInput")
    wv_d = nc.dram_tensor("wv", [C, VW], BF16, kind="ExternalInput")
    wp_d = nc.dram_tensor("wp", [C, C], BF16, kind="ExternalInput")
    bvaug_d = nc.dram_tensor("bvaug", [1, VW], BF16, kind="ExternalInput")
    ones_d = nc.dram_tensor("ones", [1, 128], BF16, kind="ExternalInput")
    vones_d = nc.dram_tensor("vones", [128, TOKC, NH], BF16, kind="ExternalInput")
    if qk_bias:
        # per-dense-row bias, column c = bias vector for chunk c's 128 rows
        bqkd_d = nc.dram_tensor("bqkd", [128, NCH], F32, kind="ExternalInput")
    if p_bias:
        bp_d = nc.dram_tensor("bp", [1, C], BF16, kind="ExternalInput")
    y_d = nc.dram_tensor("y", [BPC, N, C], F32, kind="ExternalOutput")

    xT_re = xT_d.rearrange("(o p) t -> p o t", p=128)
    wqk_re = wqk_d.rearrange("(o p) f -> p o f", p=128)
    wv_re = wv_d.rearrange("(o p) f -> p o f", p=128)
    wp_re = wp_d.rearrange("(o p) f -> p o f", p=128)

    EXP = mybir.ActivationFunctionType.Exp
    COPY = mybir.ActivationFunctionType.Copy
    MULT = mybir.AluOpType.mult

    with tile.TileContext(nc) as tc:
        with (
            tc.tile_pool(name="wpool", bufs=1) as wpool,
            tc.tile_pool(name="wqkh_pool", bufs=3) as wqkh_pool,
            tc.tile_pool(name="xpool", bufs=2) as xpool,
            tc.tile_pool(name="qktd_pool", bufs=5) as qktd_pool,
            tc.tile_pool(name="qkt_pool", bufs=4) as qkt_pool,
            tc.tile_pool(name="vpool", bufs=2) as vpool,
            tc.tile_pool(name="pt_pool", bufs=3) as pt_pool,
            tc.tile_pool(name="rbn_pool", bufs=2) as rbn_pool,
            tc.tile_pool(name="onat_pool", bufs=2) as onat_pool,
            tc.tile_pool(name="attn_pool", bufs=2) as attn_pool,
            tc.tile_pool(name="out_pool", bufs=2) as out_pool,
            tc.tile_pool(name="spsum", bufs=2, space="PSUM") as spsum,
            tc.tile_pool(name="opsum_pool", bufs=2, space="PSUM") as opsum_pool,
            tc.tile_pool(name="mpsum", bufs=2, space="PSUM") as mpsum,
        ):
            # --- resident weights/constants ---
            wv_sb = wpool.tile([128, CC, VW], BF16, tag="wv")
            for cc in range(CC):
                nc.scalar.dma_start(wv_sb[:, cc, 0:512], wv_re[:, cc, 0:512])
            nc.scalar.dma_start(wv_sb[:, :, 512:VW], wv_re[:, :, 512:VW])
            # wp is not needed until the first output projection; issued after
            # batch 1's x staging below.
            wp_sb = wpool.tile([128, CC, C], BF16, tag="wp")
            bvaug_sb = wpool.tile([1, VW], BF16, tag="bvaug")
            nc.scalar.dma_start(bvaug_sb[:], bvaug_d[:])
            ones_sb = wpool.tile([1, 128], BF16, tag="ones")
            nc.scalar.dma_start(ones_sb[:], ones_d[:])
            if qk_bias:
                bqkd_sb = wpool.tile([128, NCH], F32, tag="bqkd")
                nc.scalar.dma_start(bqkd_sb[:], bqkd_d[:])
            if p_bias:
                bp_sb = wpool.tile([1, C], BF16, tag="bp")
                nc.scalar.dma_start(bp_sb[:], bp_d[:])

            def emit_vproj(b):
                """Stage batch b's x^T and project V (ones-augmented)."""
                xTb = xpool.tile([128, CC, N], BF16, tag="xTb", name="xTb")
                for half in range(2):
                    for cc in range(CC):
                        nc.sync.dma_start(
                            xTb[:, cc, half * 512 : (half + 1) * 512],
                            xT_re[:, cc, b * N + half * 512 : b * N + (half + 1) * 512],
                        )
                v_sb = vpool.tile([128, TOKC, VW], BF16, tag="v", name="v_sb")
                v_bias = bool(qk_bias)  # b_qkv nonzero => v bias nonzero path
                for t in range(TOKC):
                    for lo, hi in ((0, 512), (512, VW)):
                        vps = mpsum.tile([128, 512], F32, tag="mpsum", name="vps")
                        w = hi - lo
                        for cc in range(CC):
                            nc.tensor.matmul(
                                vps[:, :w],
                                xTb[:, cc, t * 128 : (t + 1) * 128],
                                wv_sb[:, cc, lo:hi],
                                start=(cc == 0),
                                stop=(cc == CC - 1 and not v_bias),
                            )
                        if v_bias:
                            # bias + per-head ones-columns via rank-1 update
                            nc.tensor.matmul(
                                vps[:, :w],
                                ones_sb[:, :],
                                bvaug_sb[:, lo:hi],
                                start=False,
                                stop=True,
                            )
                        nc.vector.tensor_copy(v_sb[:, t, lo:hi], vps[:, :w])
                if not v_bias:
                    # fill each head's ones-column with a single strided DMA
                    nc.sync.dma_start(
                        v_sb.rearrange("p t (h a) -> p t h a", a=HDA)[:, :, :, HD],
                        vones_d[:],
                    )
                return xTb, v_sb

            def make_qkchunks(b, xTb):
                """Dense Q^T/K^T projection for batch b.

                Returns (drive, src) where drive(n) emits up to n more
                emission steps (half-chunk projection groups + realign DMAs)
                and src(h, f) -> AP of head-tile (h, f) as [96, N] rows at
                partition 0 (either a realigned tile or a direct qktd view).
                """
                qktd = {}  # chunk -> tile [128, N]
                qkt_tiles = {}  # h -> tile [96, 2, N]
                srcs = {}

                # (h, f) tiles completing at chunk c (i.e. last row in c)
                finish = {c: [] for c in range(NCH)}
                for h in range(NH):
                    for f in range(2):
                        o, p0, len1, len2 = _qk_tile_geom(h, f)
                        finish[o + (1 if len2 else 0)].append((h, f))

                def steps():
                    for c in range(NCH):
                        wqkh = wqkh_pool.tile([128, CC, 128], BF16, tag="wqkh", name="wqkh")
                        nc.scalar.dma_start(
                            wqkh[:], wqk_re[:, :, 128 * c : 128 * (c + 1)]
                        )
                        qktd_c = qktd_pool.tile([128, N], BF16, tag="qktd", name="qktd")
                        qktd[c] = qktd_c
                        for tq in range(QH):
                            qps = mpsum.tile([128, 512], F32, tag="mpsum", name="qps")
                            for cc in range(CC):
                                nc.tensor.matmul(
                                    qps[:, :],
                                    wqkh[:, cc, :],
                                    xTb[:, cc, tq * 512 : (tq + 1) * 512],
                                    start=(cc == 0),
                                    stop=(cc == CC - 1),
                                )
                            dst = qktd_c[:, tq * 512 : (tq + 1) * 512]
                            if qk_bias:
                                nc.scalar.activation(
                                    dst, qps[:, :], COPY, bias=bqkd_sb[:, c : c + 1]
                                )
                            else:
                                nc.vector.tensor_copy(dst, qps[:, :])
                            yield True
                        # realign head tiles finishing with this chunk
                        for h, f in finish[c]:
                            o, p0, len1, len2 = _qk_tile_geom(h, f)
                            if p0 == 0:
                                srcs[(h, f)] = qktd[o][0:HD, :]
                                continue
                            if h not in qkt_tiles:
                                qkt_tiles[h] = qkt_pool.tile(
                                    [HD, 2, N], BF16, tag="qkt", name="qkt"
                                )
                            qt = qkt_tiles[h]
                            nc.sync.dma_start(
                                qt[0:len1, f, :], qktd[o][p0 : p0 + len1, :]
                            )
                            if len2:
                                nc.sync.dma_start(
                                    qt[len1:HD, f, :], qktd[o + 1][0:len2, :]
                                )
                            srcs[(h, f)] = qt[:, f, :]
                        yield True

                gen = steps()

                def drive(n):
                    for _ in range(n):
                        if next(gen, None) is None:
                            break

                return drive, lambda h, f: srcs[(h, f)]

            def emit_heads(b, v_sb, drive, src):
                """Attention for all heads; writes normalized O into oNat
                ([q, (h, hd)] per q-tile) and returns the transposed attnT."""
                oNat = onat_pool.tile([128, TOKC, C], BF16, tag="oNat", name="oNat")
                for h in range(NH):
                    srcQ = src(h, 0)
                    srcK = src(h, 1)
                    ops = opsum_pool.tile(
                        [128, TOKC, 128], F32, tag="opsum", name="ops"
                    )
                    for kc in range(KC):
                        pt = pt_pool.tile([128, QH, 512], BF16, tag="pt", name="pt")
                        for qh in range(QH):
                            st = spsum.tile([128, 512], F32, tag="spsum", name="st")
                            nc.tensor.matmul(
                                st[:, :],
                                srcK[:, kc * 128 : (kc + 1) * 128],
                                srcQ[:, qh * 512 : (qh + 1) * 512],
                                start=True,
                                stop=True,
                            )
                            nc.scalar.activation(
                                pt[:, qh, :], st[:, :], EXP, scale=SCALE
                            )
                        # keep the dense projection pipeline fed
                        drive(1)
                        for qt in range(TOKC):
                            qh, qi = divmod(qt, 4)
                            # PSUM start=True clears has_written for the WHOLE
                            # bank, so only the first q-tile region per bank
                            # may use it; the others rely on flags=0
                            # overwrite-where-unwritten for their first write.
                            nc.tensor.matmul(
                                ops[:, qt, 0:HDA],
                                pt[:, qh, qi * 128 : (qi + 1) * 128],
                                v_sb[:, kc, HDA * h : HDA * h + HDA],
                                start=(kc == 0 and qt % 4 == 0),
                                stop=(kc == KC - 1),
                                skip_group_check=True,
                            )
                    # normalize: per-partition reciprocal row-sum, broadcast
                    # along the free dim by tensor_scalar
                    rbn = rbn_pool.tile([128, TOKC], F32, tag="rbn", name="rbn")
                    nc.vector.reciprocal(rbn[:, :], ops[:, :, HD])
                    for qt in range(TOKC):
                        nc.vector.tensor_scalar(
                            oNat[:, qt, HD * h : HD * (h + 1)],
                            ops[:, qt, 0:HD],
                            rbn[:, qt : qt + 1],
                            None,
                            MULT,
                        )
                # repack to [C, q] layout: one XBAR transpose per q-tile
                attnT = attn_pool.tile(
                    [128, QC, CC, 128], BF16, tag="attnT", name="attnT"
                )
                for qt in range(TOKC):
                    nc.scalar.dma_start_transpose(
                        attnT[:, qt, :, :], oNat[:, qt, :]
                    )
                return attnT

            def emit_proj(b, attnT):
                """Output projection: dense 6-chunk contraction."""
                for qc in range(QC):
                    out_sb = out_pool.tile([128, C], F32, tag="out", name="out_sb")
                    for n in range(2):
                        pps = mpsum.tile([128, 512], F32, tag="mpsum", name="pps")
                        ns = slice(n * 384, (n + 1) * 384)
                        for cc in range(CC):
                            nc.tensor.matmul(
                                pps[:, :384],
                                attnT[:, qc, cc, :],
                                wp_sb[:, cc, ns],
                                start=(cc == 0),
                                stop=(cc == CC - 1 and not p_bias),
                            )
                        if p_bias:
                            nc.tensor.matmul(
                                pps[:, :384],
                                ones_sb[:, :],
                                bp_sb[:, ns],
                                start=False,
                                stop=True,
                            )
                        if n == 0:
                            nc.scalar.activation(out_sb[:, ns], pps[:, :384], COPY)
                        else:
                            nc.vector.tensor_copy(out_sb[:, ns], pps[:, :384])
                    nc.sync.dma_start(y_d[b, qc * 128 : (qc + 1) * 128, :], out_sb[:])

            # Emission order V0 G0 H0 V1 G1 H1 P0 P1: batch 1's V projection
            # fills the PE while batch 0's normalize tail drains, and proj(0)
            # fills it while batch 1's first heads project.
            xTb0, v0 = emit_vproj(0)
            drive0, src0 = make_qkchunks(0, xTb0)
            drive0(6)  # chunks 0-1 (+realign) must precede head 0's scores
            at0 = emit_heads(0, v0, drive0, src0)
            drive0(99)  # flush any remainder
            xTb1, v1 = emit_vproj(1)
            nc.scalar.dma_start(wp_sb[:], wp_re[:])
            drive1, src1 = make_qkchunks(1, xTb1)
            drive1(6)
            at1 = emit_heads(1, v1, drive1, src1)
            drive1(99)
            emit_proj(0, at0)
            emit_proj(1, at1)

    nc.compile()
    _BUILD_CACHE[key] = nc
    return nc


def _prep_shared(w_qkv, b_qkv, w_proj, b_proj):
    """Host-side weight rearrangement shared by all cores."""
    w_qkv = np.ascontiguousarray(w_qkv, dtype=np.float32)
    w_proj = np.ascontiguousarray(w_proj, dtype=np.float32)
    b_qkv = np.asarray(b_qkv, dtype=np.float32)
    b_proj = np.asarray(b_proj, dtype=np.float32)

    # wqk: [C, 2*NH*HD] with column 2*HD*h + HD*f + j = w_qkv row C*f + HD*h + j
    wqk = w_qkv[: 2 * C].reshape(2, NH, HD, C)  # [f, h, j, c]
    wqk_arr = np.ascontiguousarray(
        np.transpose(wqk, (3, 1, 0, 2)).reshape(C, 2 * NH * HD).astype(NPBF16)
    )

    # wv: [C, NH*(HD+1)] with a zero ones-column slot per head
    wv = w_qkv[2 * C :].reshape(NH, HD, C)  # [h, j, c]
    wv_aug = np.zeros((C, NH, HDA), dtype=NPBF16)
    wv_aug[:, :, :HD] = np.transpose(wv, (2, 0, 1)).astype(NPBF16)
    wv_aug = np.ascontiguousarray(wv_aug.reshape(C, VW))

    # wp: plain transpose [c_in, c_out]
    wp_t = np.ascontiguousarray(w_proj.T.astype(NPBF16))

    # bvaug: v-bias interleaved with 1.0 at each head's ones-column
    bvaug = np.zeros((1, NH, HDA), dtype=np.float32)
    bvaug[0, :, :HD] = b_qkv[2 * C :].reshape(NH, HD)
    bvaug[0, :, HD] = 1.0
    bvaug = bvaug.reshape(1, VW).astype(NPBF16)

    ones = np.ones((1, 128), dtype=NPBF16)
    vones = np.ones((128, TOKC, NH), dtype=NPBF16)

    qk_bias = bool(np.any(b_qkv[: 2 * C] != 0.0))
    p_bias = bool(np.any(b_proj != 0.0))
    extra = {}
    if qk_bias:
        # dense-row order (h, f, j), reshaped so column c = chunk c's rows
        bqk = b_qkv[: 2 * C].reshape(2, NH, HD)  # [f, h, j]
        dense = np.transpose(bqk, (1, 0, 2)).reshape(2 * NH * HD)
        extra["bqkd"] = np.ascontiguousarray(dense.reshape(NCH, 128).T)
    if p_bias:
        extra["bp"] = np.ascontiguousarray(b_proj.reshape(1, C).astype(NPBF16))

    return wqk_arr, wv_aug, wp_t, bvaug, ones, vones, qk_bias, p_bias, extra


def kernel(x, w_qkv, b_qkv, w_proj, b_proj, H=32, W=32):
    x = np.asarray(x, dtype=np.float32)
    assert x.shape == (B, N, C), x.shape
    assert int(H) * int(W) == N

    wqk_arr, wv_aug, wp_t, bvaug, ones, vones, qk_bias, p_bias, extra = _prep_shared(
        w_qkv, b_qkv, w_proj, b_proj
    )
    nc = _build(qk_bias, p_bias)

    in_maps = []
    for c in range(NCORES):
        xc = x[BPC * c : BPC * (c + 1)].reshape(BPC * N, C)
        xT = np.ascontiguousarray(xc.T.astype(NPBF16))  # [C, BPC*N]
        m = {
            "xT": xT,
            "wqk": wqk_arr,
            "wv": wv_aug,
            "wp": wp_t,
            "bvaug": bvaug,
            "ones": ones,
            "vones": vones,
        }
        m.update(extra)
        in_maps.append(m)

    trace = os.environ.get("KERNEL_TRACE") == "1"
    res = run_bass_kernel_spmd(
        nc, in_maps, core_ids=list(range(NCORES)), trace=trace
    )
    if trace:
        kernel.last_results = res
        print("exec_time_ns:", res.exec_time_ns, "mean:", res.mean_exec_time_ns)
        if res.instructions_and_trace:
            print("trace:", res.instructions_and_trace[1])

    out = np.empty((B, N, C), dtype=np.float32)
    for c in range(NCORES):
        out[BPC * c : BPC * (c + 1)] = res.results[c]["y"]
    return out


if __name__ == "__main__":
    rng = np.random.default_rng(0)
    x = rng.standard_normal((B, N, C), dtype=np.float32)
    w_qkv = rng.standard_normal((3 * C, C), dtype=np.float32) / np.sqrt(C)
    b_qkv = np.zeros(3 * C, np.float32)
    w_proj = rng.standard_normal((C, C), dtype=np.float32) / np.sqrt(C)
    b_proj = np.zeros(C, np.float32)
    y = kernel(x, w_qkv, b_qkv, w_proj, b_proj)
    print("out", y.shape, y.dtype, float(np.abs(y).mean()))


# revision 17
# speedup vs baseline: 1.1339x; 1.0747x over previous
"""Trainium2 Bass kernel for multi-head global attention (the
"DeformableAttention" module whose relative-position-bias path is inactive).

Reference computation (per batch b):
    qkv = x @ w_qkv.T + b_qkv            # [N, 3C]
    q, k, v = split/reshape to [nh, N, hd]
    attn = softmax((q @ k.T) * hd**-0.5)
    out  = (attn @ v) merged heads       # [N, C]
    y    = out @ w_proj.T + b_proj

Sharding: data-parallel over batch B=16 across 8 NeuronCores (2 batches/core).
No collectives.

Device-side design (per core, per batch), all SBUF operands bf16 so every
matmul streams at 1 cycle/row and 128-column stationaries get fast weight
load:
  * x is staged pre-transposed (xT, [C, tokens]).
  * Q^T/K^T are produced by a DENSE projection: 12 chunks of M=128 rows in
    (head, q|k, j) order -- no M=96 underutilization. Head tiles that start
    mid-chunk are realigned to partition 0 by small SBUF->SBUF DMAs; the 4
    chunk-aligned tiles are read in place.
  * V in natural [N, nh*(hd+1)] layout with an interleaved ones-column per
    head.
  * Scores are computed transposed (S^T[k, q] blocks), softmax's exp runs on
    ScalarE with the 1/sqrt(hd) scale fused.
  * AV runs QUERY-NATURAL: out[q, hd] tiles with M=128 (full PE height),
    F=97 bf16; the ones-column row-sum lands on the same partition as its
    queries, so the normalize is a per-partition reciprocal + tensor_scalar
    (no cross-partition broadcast, no DRAM bounce).
  * The normalized per-q-tile [128, C] block (heads side by side) is turned
    into the projection's [C, q] layout by one XBAR DMA-transpose per q-tile.
  * Output projection contracts 6 dense 128-chunks, producing [token, C]
    tiles for direct DMA out.
"""

import os
import sys

sys.path.insert(0, "/opt/trn_rl_repo")

# The Bass->PJRT execution path needs jax to discover the axon-tunneled
# NeuronCores; a stray JAX_PLATFORMS=cpu (e.g. set for a jax reference run)
# would hide them. Only effective if jax hasn't been imported yet.
if "jax" not in sys.modules and "axon" not in os.environ.get("JAX_PLATFORMS", "axon"):
    os.environ.pop("JAX_PLATFORMS", None)

import ml_dtypes
import numpy as np

import concourse.bass as bass
import concourse.mybir as mybir
import concourse.tile as tile
from concourse import bacc
from concourse.bass_utils import run_bass_kernel_spmd

# Problem constants (hardcoded per the task contract).
B, N, C = 16, 1024, 768
NH, HD = 8, 96
NCORES = 8
BPC = B // NCORES  # batches per core = 2
CC = C // 128  # contraction chunks of 128 = 6
KC = N // 128  # key chunks per batch = 8
QH = N // 512  # query halves = 2
TOKC = N // 128  # token chunks = 8
QC = N // 128  # query chunks for output projection = 8
HDA = HD + 1  # head dim + ones column = 97
VW = NH * HDA  # augmented V width = 776
NCH = 2 * NH * HD // 128  # dense Q/K projection chunks = 12
SCALE = float(HD) ** -0.5

F32 = mybir.dt.float32
BF16 = mybir.dt.bfloat16
DENSE_CYC = 6 * 512  # PE cycles of one dense-projection tq step
CHUNK_CYC = 2 * DENSE_CYC + 1  # tq steps + the (1-cycle) realign step
PROJ_CYC = 6 * 384  # PE cycles of one output-projection group
NPBF16 = ml_dtypes.bfloat16

_BUILD_CACHE = {}


def _qk_tile_geom(h, f):
    """Dense-row geometry of head-tile (h, f): rows r0..r0+95 of the
    (h, f, j) row space land in chunk o at partitions p0.., possibly
    spilling len2 rows into chunk o+1."""
    r0 = 2 * HD * h + HD * f
    o, p0 = divmod(r0, 128)
    len1 = min(HD, 128 - p0)
    return o, p0, len1, HD - len1


def _build(qk_bias: bool, p_bias: bool):
    """Build + compile the single-core Bass program (shared SPMD across cores)."""
    key = (qk_bias, p_bias)
    if key in _BUILD_CACHE:
        return _BUILD_CACHE[key]

    nc = bacc.Bacc("TRN2", target_bir_lowering=False, debug=False)

    xT_d = nc.dram_tensor("xT", [C, BPC * N], BF16, kind="ExternalInput")
    wqk_d = nc.dram_tensor("wqk", [C, 2 * NH * HD], BF16, kind="ExternalInput")
    wv_d = nc.dram_tensor("wv", [C, VW], BF16, kind="ExternalInput")
    wp_d = nc.dram_tensor("wp", [C, C], BF16, kind="ExternalInput")
    bvaug_d = nc.dram_tensor("bvaug", [1, VW], BF16, kind="ExternalInput")
    ones_d = nc.dram_tensor("ones", [1, 128], BF16, kind="ExternalInput")
    vones_d = nc.dram_tensor("vones", [128, TOKC, NH], BF16, kind="ExternalInput")
    if qk_bias:
        # per-dense-row bias, column c = bias vector for chunk c's 128 rows
        bqkd_d = nc.dram_tensor("bqkd", [128, NCH], F32, kind="ExternalInput")
    if p_bias:
        bp_d = nc.dram_tensor("bp", [1, C], BF16, kind="ExternalInput")
    y_d = nc.dram_tensor("y", [BPC, N, C], F32, kind="ExternalOutput")

    xT_re = xT_d.rearrange("(o p) t -> p o t", p=128)
    wqk_re = wqk_d.rearrange("(o p) f -> p o f", p=128)
    wv_re = wv_d.rearrange("(o p) f -> p o f", p=128)
    wp_re = wp_d.rearrange("(o p) f -> p o f", p=128)

    EXP = mybir.ActivationFunctionType.Exp
    COPY = mybir.ActivationFunctionType.Copy
    MULT = mybir.AluOpType.mult

    with tile.TileContext(nc) as tc:
        with (
            tc.tile_pool(name="wpool", bufs=1) as wpool,
            tc.tile_pool(name="wqkh_pool", bufs=3) as wqkh_pool,
            tc.tile_pool(name="xpool", bufs=2) as xpool,
            tc.tile_pool(name="qktd_pool", bufs=5) as qktd_pool,
            tc.tile_pool(name="qkt_pool", bufs=4) as qkt_pool,
            tc.tile_pool(name="vpool", bufs=2) as vpool,
            tc.tile_pool(name="pt_pool", bufs=3) as pt_pool,
            tc.tile_pool(name="rbn_pool", bufs=2) as rbn_pool,
            tc.tile_pool(name="onat_pool", bufs=2) as onat_pool,
            tc.tile_pool(name="attn_pool", bufs=2) as attn_pool,
            tc.tile_pool(name="out_pool", bufs=2) as out_pool,
            tc.tile_pool(name="spsum", bufs=2, space="PSUM") as spsum,
            tc.tile_pool(name="opsum_pool", bufs=2, space="PSUM") as opsum_pool,
            tc.tile_pool(name="mpsum", bufs=2, space="PSUM") as mpsum,
        ):
            # --- resident weights/constants ---
            wv_sb = wpool.tile([128, CC, VW], BF16, tag="wv")
            nc.scalar.dma_start(wv_sb[:, :, 0:512], wv_re[:, :, 0:512])
            nc.scalar.dma_start(wv_sb[:, :, 512:VW], wv_re[:, :, 512:VW])
            # wp is not needed until the first output projection; issued after
            # batch 1's x staging below.
            wp_sb = wpool.tile([128, CC, C], BF16, tag="wp")
            bvaug_sb = wpool.tile([1, VW], BF16, tag="bvaug")
            nc.scalar.dma_start(bvaug_sb[:], bvaug_d[:])
            ones_sb = wpool.tile([1, 128], BF16, tag="ones")
            nc.scalar.dma_start(ones_sb[:], ones_d[:])
            if qk_bias:
                bqkd_sb = wpool.tile([128, NCH], F32, tag="bqkd")
                nc.scalar.dma_start(bqkd_sb[:], bqkd_d[:])
            if p_bias:
                bp_sb = wpool.tile([1, C], BF16, tag="bp")
                nc.scalar.dma_start(bp_sb[:], bp_d[:])

            def stage_x(b, split_first=False):
                """Issue batch b's x^T staging DMAs (2-3 big descriptors).
                split_first carves out the first token tile so the very first
                V-projection group can start before the rest of x lands."""
                xTb = xpool.tile([128, CC, N], BF16, tag="xTb", name="xTb")
                pieces = ((0, 128), (128, 512), (512, N)) if split_first else (
                    (0, 512), (512, N))
                for lo, hi in pieces:
                    nc.sync.dma_start(
                        xTb[:, :, lo:hi],
                        xT_re[:, :, b * N + lo : b * N + hi],
                    )
                return xTb

            def gen_v(b, xTb, v_sb):
                """V projection steps (16 PSUM groups) for batch b."""
                v_bias = bool(qk_bias)  # b_qkv nonzero => v bias nonzero path
                for lo, hi in ((0, 512), (512, VW)):
                    for t in range(TOKC):
                        vps = mpsum.tile([128, 512], F32, tag="mpsum", name="vps")
                        w = hi - lo
                        for cc in range(CC):
                            nc.tensor.matmul(
                                vps[:, :w],
                                xTb[:, cc, t * 128 : (t + 1) * 128],
                                wv_sb[:, cc, lo:hi],
                                start=(cc == 0),
                                stop=(cc == CC - 1 and not v_bias),
                            )
                        if v_bias:
                            # bias + per-head ones-columns via rank-1 update
                            nc.tensor.matmul(
                                vps[:, :w],
                                ones_sb[:, :],
                                bvaug_sb[:, lo:hi],
                                start=False,
                                stop=True,
                            )
                        nc.vector.tensor_copy(v_sb[:, t, lo:hi], vps[:, :w])
                        yield CC * w
                if not v_bias:
                    # fill each head's ones-column with a single strided DMA
                    nc.sync.dma_start(
                        v_sb.rearrange("p t (h a) -> p t h a", a=HDA)[:, :, :, HD],
                        vones_d[:],
                    )

            def make_qkchunks(b, xTb):
                """Dense Q^T/K^T projection for batch b.

                Returns (gen, src): gen yields after each emission step
                (half-chunk projection group or realign-DMA bundle; 3 steps
                per chunk, 36 total) and src(h, f) -> AP of head-tile (h, f)
                as [96, N] rows at partition 0 (either a realigned tile or a
                direct qktd view).
                """
                qktd = {}  # chunk -> tile [128, N]
                qkt_tiles = {}  # h -> tile [96, 2, N]
                srcs = {}
                wqkh_groups = {}  # g -> tile [128, CC, 384] (chunks 3g..3g+2)

                # (h, f) tiles completing at chunk c (i.e. last row in c)
                finish = {c: [] for c in range(NCH)}
                for h in range(NH):
                    for f in range(2):
                        o, p0, len1, len2 = _qk_tile_geom(h, f)
                        finish[o + (1 if len2 else 0)].append((h, f))

                def load_group(g):
                    wqkh = wqkh_pool.tile(
                        [128, CC, 384], BF16, tag="wqkh", name="wqkh"
                    )
                    nc.sync.dma_start(
                        wqkh[:], wqk_re[:, :, 384 * g : 384 * (g + 1)]
                    )
                    wqkh_groups[g] = wqkh

                def steps():
                    load_group(0)
                    for c in range(NCH):
                        if c % 3 == 0 and (c // 3) + 1 < NCH // 3:
                            load_group(c // 3 + 1)  # prefetch next group
                        wqkh = wqkh_groups[c // 3]
                        ws = slice((c % 3) * 128, (c % 3 + 1) * 128)
                        qktd_c = qktd_pool.tile([128, N], BF16, tag="qktd", name="qktd")
                        qktd[c] = qktd_c
                        for tq in range(QH):
                            qps = mpsum.tile([128, 512], F32, tag="mpsum", name="qps")
                            for cc in range(CC):
                                nc.tensor.matmul(
                                    qps[:, :],
                                    wqkh[:, cc, ws],
                                    xTb[:, cc, tq * 512 : (tq + 1) * 512],
                                    start=(cc == 0),
                                    stop=(cc == CC - 1),
                                )
                            dst = qktd_c[:, tq * 512 : (tq + 1) * 512]
                            if qk_bias:
                                nc.scalar.activation(
                                    dst, qps[:, :], COPY, bias=bqkd_sb[:, c : c + 1]
                                )
                            else:
                                nc.vector.tensor_copy(dst, qps[:, :])
                            yield DENSE_CYC
                        # realign head tiles finishing with this chunk
                        for h, f in finish[c]:
                            o, p0, len1, len2 = _qk_tile_geom(h, f)
                            if p0 == 0:
                                srcs[(h, f)] = qktd[o][0:HD, :]
                                continue
                            if h not in qkt_tiles:
                                qkt_tiles[h] = qkt_pool.tile(
                                    [HD, 2, N], BF16, tag="qkt", name="qkt"
                                )
                            qt = qkt_tiles[h]
                            nc.sync.dma_start(
                                qt[0:len1, f, :], qktd[o][p0 : p0 + len1, :]
                            )
                            if len2:
                                nc.sync.dma_start(
                                    qt[len1:HD, f, :], qktd[o + 1][0:len2, :]
                                )
                            srcs[(h, f)] = qt[:, f, :]
                        yield 1

                return steps(), lambda h, f: srcs[(h, f)]

            # dense-projection PE-cycles that must be complete before head
            # h's scores: 2 tq steps per chunk, through the chunk holding the
            # last row of tile (h, f=1)
            def dense_need(h):
                return CHUNK_CYC * ((2 * HD * h + HD + HD - 1) // 128 + 1)

            class Driver:
                """Drains a chain of filler generators into the attention
                loop's PE-idle windows, paced fractionally by PE cycles."""

                def __init__(self, gens, total):
                    self.gens = list(gens)
                    self.total = total
                    self.driven = 0

                def drive_to(self, target):
                    target = min(target, self.total)
                    while self.driven < target and self.gens:
                        v = next(self.gens[0], None)
                        if v is None:
                            self.gens.pop(0)
                            continue
                        self.driven += v

                def finish(self):
                    self.drive_to(self.total)

            def emit_heads(b, v_sb, driver, src, pre_driven):
                """Attention for all heads; writes normalized O into oNat
                ([q, (h, hd)] per q-tile) and returns the transposed attnT.

                driver's filler chain starts with this batch's remaining
                dense-projection steps (dense_need deadlines are enforced
                relative to pre_driven)."""
                oNat = onat_pool.tile([128, TOKC, C], BF16, tag="oNat", name="oNat")
                for h in range(NH):
                    driver.drive_to(dense_need(h) - pre_driven)
                    srcQ = src(h, 0)
                    srcK = src(h, 1)
                    ops = opsum_pool.tile(
                        [128, TOKC, 128], F32, tag="opsum", name="ops"
                    )
                    def emit_av(kc, pt):
                        for qt in range(TOKC):
                            qh, qi = divmod(qt, 4)
                            # PSUM start=True clears has_written for the WHOLE
                            # bank, so only the first q-tile region per bank
                            # may use it; the others rely on flags=0
                            # overwrite-where-unwritten for their first write.
                            nc.tensor.matmul(
                                ops[:, qt, 0:HDA],
                                pt[:, qh, qi * 128 : (qi + 1) * 128],
                                v_sb[:, kc, HDA * h : HDA * h + HDA],
                                start=(kc == 0 and qt % 4 == 0),
                                stop=(kc == KC - 1),
                                skip_group_check=True,
                            )

                    prev_pt = None
                    for kc in range(KC):
                        pt = pt_pool.tile([128, QH, 512], BF16, tag="pt", name="pt")
                        for qh in range(QH):
                            st = spsum.tile([128, 512], F32, tag="spsum", name="st")
                            nc.tensor.matmul(
                                st[:, :],
                                srcK[:, kc * 128 : (kc + 1) * 128],
                                srcQ[:, qh * 512 : (qh + 1) * 512],
                                start=True,
                                stop=True,
                            )
                            nc.scalar.activation(
                                pt[:, qh, :], st[:, :], EXP, scale=SCALE
                            )
                        # keep the PE fed through the exp latency: fractional
                        # pacing of the filler chain across all 64 kc-steps,
                        # and run the PREVIOUS kc's AV (its exp is long done)
                        idx = 8 * h + kc
                        driver.drive_to(-(-driver.total * (idx + 1) // 64))
                        if prev_pt is not None:
                            emit_av(kc - 1, prev_pt)
                        prev_pt = pt
                    emit_av(KC - 1, prev_pt)
                    # normalize: per-partition reciprocal row-sum, broadcast
                    # along the free dim by tensor_scalar
                    rbn = rbn_pool.tile([128, TOKC], F32, tag="rbn", name="rbn")
                    nc.vector.reciprocal(rbn[:, :], ops[:, :, HD])
                    for qt in range(TOKC):
                        nc.vector.tensor_scalar(
                            oNat[:, qt, HD * h : HD * (h + 1)],
                            ops[:, qt, 0:HD],
                            rbn[:, qt : qt + 1],
                            None,
                            MULT,
                        )
                # repack to [C, q] layout: one XBAR transpose per q-tile
                attnT = attn_pool.tile(
                    [128, QC, CC, 128], BF16, tag="attnT", name="attnT"
                )
                for qt in range(TOKC):
                    nc.sync.dma_start_transpose(
                        attnT[:, qt, :, :], oNat[:, qt, :]
                    )
                return attnT

            def gen_proj(b, attnT, tail=False):
                """Output projection steps (32 PSUM groups)."""
                for qc in range(QC):
                    out_sb = out_pool.tile([128, C], F32, tag="out", name="out_sb")
                    for n in range(2):
                        pps = mpsum.tile([128, 512], F32, tag="mpsum", name="pps")
                        ns = slice(n * 384, (n + 1) * 384)
                        for cc in range(CC):
                            nc.tensor.matmul(
                                pps[:, :384],
                                attnT[:, qc, cc, :],
                                wp_sb[:, cc, ns],
                                start=(cc == 0),
                                stop=(cc == CC - 1 and not p_bias),
                            )
                        if p_bias:
                            nc.tensor.matmul(
                                pps[:, :384],
                                ones_sb[:, :],
                                bp_sb[:, ns],
                                start=False,
                                stop=True,
                            )
                        if n == 0 and tail:
                            nc.scalar.activation(out_sb[:, ns], pps[:, :384], COPY)
                        else:
                            nc.vector.tensor_copy(out_sb[:, ns], pps[:, :384])
                        yield PROJ_CYC
                    nc.sync.dma_start(y_d[b, qc * 128 : (qc + 1) * 128, :], out_sb[:])

            def take(gen, n):
                for _ in range(n):
                    v = next(gen, None)
                    if v is None:
                        return
                    yield v

            # Emission plan: V0 runs unoverlapped (nothing precedes it); the
            # attention kc-loops of batch 0 absorb [dense-rest(0), V(1),
            # dense-pre(1)] as PE filler; batch 1's absorb [dense-rest(1),
            # proj(0)]; proj(1) is the tail.
            xTb0 = stage_x(0, split_first=True)
            v0 = vpool.tile([128, TOKC, VW], BF16, tag="v", name="v0")
            for _ in gen_v(0, xTb0, v0):
                pass
            g0, src0 = make_qkchunks(0, xTb0)
            # chunks 0-1 (+realign) must precede head 0's scores
            Driver([g0], 2 * CHUNK_CYC).finish()
            xTb1 = stage_x(1)
            nc.scalar.dma_start(wp_sb[:], wp_re[:])
            v1 = vpool.tile([128, TOKC, VW], BF16, tag="v", name="v1")
            g1, src1 = make_qkchunks(1, xTb1)
            V_CYC = 8 * CC * 512 + 8 * CC * 264
            drv0 = Driver(
                [g0, gen_v(1, xTb1, v1), take(g1, 6)],
                10 * CHUNK_CYC + V_CYC + 2 * CHUNK_CYC,
            )
            at0 = emit_heads(0, v0, drv0, src0, pre_driven=2 * CHUNK_CYC)
            drv0.finish()
            drv1 = Driver(
                [g1, gen_proj(0, at0)], 10 * CHUNK_CYC + 32 * PROJ_CYC
            )
            at1 = emit_heads(1, v1, drv1, src1, pre_driven=2 * CHUNK_CYC)
            drv1.finish()
            for _ in gen_proj(1, at1, tail=True):
                pass

    nc.compile()
    _BUILD_CACHE[key] = nc
    return nc


def _prep_shared(w_qkv, b_qkv, w_proj, b_proj):
    """Host-side weight rearrangement shared by all cores."""
    w_qkv = np.ascontiguousarray(w_qkv, dtype=np.float32)
    w_proj = np.ascontiguousarray(w_proj, dtype=np.float32)
    b_qkv = np.asarray(b_qkv, dtype=np.float32)
    b_proj = np.asarray(b_proj, dtype=np.float32)

    # wqk: [C, 2*NH*HD] with column 2*HD*h + HD*f + j = w_qkv row C*f + HD*h + j
    wqk = w_qkv[: 2 * C].reshape(2, NH, HD, C)  # [f, h, j, c]
    wqk_arr = np.ascontiguousarray(
        np.transpose(wqk, (3, 1, 0, 2)).reshape(C, 2 * NH * HD).astype(NPBF16)
    )

    # wv: [C, NH*(HD+1)] with a zero ones-column slot per head
    wv = w_qkv[2 * C :].reshape(NH, HD, C)  # [h, j, c]
    wv_aug = np.zeros((C, NH, HDA), dtype=NPBF16)
    wv_aug[:, :, :HD] = np.transpose(wv, (2, 0, 1)).astype(NPBF16)
    wv_aug = np.ascontiguousarray(wv_aug.reshape(C, VW))

    # wp: plain transpose [c_in, c_out]
    wp_t = np.ascontiguousarray(w_proj.T.astype(NPBF16))

    # bvaug: v-bias interleaved with 1.0 at each head's ones-column
    bvaug = np.zeros((1, NH, HDA), dtype=np.float32)
    bvaug[0, :, :HD] = b_qkv[2 * C :].reshape(NH, HD)
    bvaug[0, :, HD] = 1.0
    bvaug = bvaug.reshape(1, VW).astype(NPBF16)

    ones = np.ones((1, 128), dtype=NPBF16)
    vones = np.ones((128, TOKC, NH), dtype=NPBF16)

    qk_bias = bool(np.any(b_qkv[: 2 * C] != 0.0))
    p_bias = bool(np.any(b_proj != 0.0))
    extra = {}
    if qk_bias:
        # dense-row order (h, f, j), reshaped so column c = chunk c's rows
        bqk = b_qkv[: 2 * C].reshape(2, NH, HD)  # [f, h, j]
        dense = np.transpose(bqk, (1, 0, 2)).reshape(2 * NH * HD)
        extra["bqkd"] = np.ascontiguousarray(dense.reshape(NCH, 128).T)
    if p_bias:
        extra["bp"] = np.ascontiguousarray(b_proj.reshape(1, C).astype(NPBF16))

    return wqk_arr, wv_aug, wp_t, bvaug, ones, vones, qk_bias, p_bias, extra


def kernel(x, w_qkv, b_qkv, w_proj, b_proj, H=32, W=32):
    x = np.asarray(x, dtype=np.float32)
    assert x.shape == (B, N, C), x.shape
    assert int(H) * int(W) == N

    wqk_arr, wv_aug, wp_t, bvaug, ones, vones, qk_bias, p_bias, extra = _prep_shared(
        w_qkv, b_qkv, w_proj, b_proj
    )
    nc = _build(qk_bias, p_bias)

    in_maps = []
    for c in range(NCORES):
        xc = x[BPC * c : BPC * (c + 1)].reshape(BPC * N, C)
        xT = np.ascontiguousarray(xc.T.astype(NPBF16))  # [C, BPC*N]
        m = {
            "xT": xT,
            "wqk": wqk_arr,
            "wv": wv_aug,
            "wp": wp_t,
            "bvaug": bvaug,
            "ones": ones,
            "vones": vones,
        }
        m.update(extra)
        in_maps.append(m)

    trace = os.environ.get("KERNEL_TRACE") == "1"
    res = run_bass_kernel_spmd(
        nc, in_maps, core_ids=list(range(NCORES)), trace=trace
    )
    if trace:
        kernel.last_results = res
        print("exec_time_ns:", res.exec_time_ns, "mean:", res.mean_exec_time_ns)
        if res.instructions_and_trace:
            print("trace:", res.instructions_and_trace[1])

    out = np.empty((B, N, C), dtype=np.float32)
    for c in range(NCORES):
        out[BPC * c : BPC * (c + 1)] = res.results[c]["y"]
    return out


if __name__ == "__main__":
    rng = np.random.default_rng(0)
    x = rng.standard_normal((B, N, C), dtype=np.float32)
    w_qkv = rng.standard_normal((3 * C, C), dtype=np.float32) / np.sqrt(C)
    b_qkv = np.zeros(3 * C, np.float32)
    w_proj = rng.standard_normal((C, C), dtype=np.float32) / np.sqrt(C)
    b_proj = np.zeros(C, np.float32)
    y = kernel(x, w_qkv, b_qkv, w_proj, b_proj)
    print("out", y.shape, y.dtype, float(np.abs(y).mean()))


# revision 27
# speedup vs baseline: 1.2122x; 1.0691x over previous
"""Trainium2 Bass kernel for multi-head global attention (the
"DeformableAttention" module whose relative-position-bias path is inactive).

Reference computation (per batch b):
    qkv = x @ w_qkv.T + b_qkv            # [N, 3C]
    q, k, v = split/reshape to [nh, N, hd]
    attn = softmax((q @ k.T) * hd**-0.5)
    out  = (attn @ v) merged heads       # [N, C]
    y    = out @ w_proj.T + b_proj

Sharding: data-parallel over batch B=16 across 8 NeuronCores (2 batches/core).
No collectives.

Device-side design (per core, per batch), all SBUF operands bf16 so every
matmul streams at 1 cycle/row and 128-column stationaries get fast weight
load:
  * x is staged pre-transposed (xT, [C, tokens]).
  * Q^T/K^T are produced by a DENSE projection: 12 chunks of M=128 rows in
    (head, q|k, j) order -- no M=96 underutilization. Head tiles that start
    mid-chunk are realigned to partition 0 by small SBUF->SBUF DMAs; the 4
    chunk-aligned tiles are read in place.
  * V in natural [N, nh*(hd+1)] layout with an interleaved ones-column per
    head.
  * Scores are computed transposed (S^T[k, q] blocks), softmax's exp runs on
    ScalarE with the 1/sqrt(hd) scale fused.
  * AV runs QUERY-NATURAL: out[q, hd] tiles with M=128 (full PE height),
    F=97 bf16; the ones-column row-sum lands on the same partition as its
    queries, so the normalize is a per-partition reciprocal + tensor_scalar
    (no cross-partition broadcast, no DRAM bounce).
  * The normalized per-q-tile [128, C] block (heads side by side) is turned
    into the projection's [C, q] layout by one XBAR DMA-transpose per q-tile.
  * Output projection contracts 6 dense 128-chunks, producing [token, C]
    tiles for direct DMA out.
"""

import os
import sys

sys.path.insert(0, "/opt/trn_rl_repo")

# The Bass->PJRT execution path needs jax to discover the axon-tunneled
# NeuronCores; a stray JAX_PLATFORMS=cpu (e.g. set for a jax reference run)
# would hide them. Only effective if jax hasn't been imported yet.
if "jax" not in sys.modules and "axon" not in os.environ.get("JAX_PLATFORMS", "axon"):
    os.environ.pop("JAX_PLATFORMS", None)

import ml_dtypes
import numpy as np

import concourse.bass as bass
import concourse.mybir as mybir
import concourse.tile as tile
from concourse import bacc
from concourse.bass_utils import run_bass_kernel_spmd

# Problem constants (hardcoded per the task contract).
B, N, C = 16, 1024, 768
NH, HD = 8, 96
NCORES = 8
BPC = B // NCORES  # batches per core = 2
CC = C // 128  # contraction chunks of 128 = 6
KC = N // 128  # key chunks per batch = 8
QH = N // 512  # query halves = 2
TOKC = N // 128  # token chunks = 8
QC = N // 128  # query chunks for output projection = 8
HDA = HD + 1  # head dim + ones column = 97
VW = NH * HDA  # augmented V width = 776
NCH = 2 * NH * HD // 128  # dense Q/K projection chunks = 12
SCALE = float(HD) ** -0.5

F32 = mybir.dt.float32
BF16 = mybir.dt.bfloat16
DENSE_CYC = 6 * 512  # PE cycles of one dense-projection tq step
CHUNK_CYC = 2 * DENSE_CYC + 1  # tq steps + the (1-cycle) realign step
PROJ_CYC = 6 * 384  # PE cycles of one output-projection group
NPBF16 = ml_dtypes.bfloat16

_BUILD_CACHE = {}


def _qk_tile_geom(h, f):
    """Dense-row geometry of head-tile (h, f): rows r0..r0+95 of the
    (h, f, j) row space land in chunk o at partitions p0.., possibly
    spilling len2 rows into chunk o+1."""
    r0 = 2 * HD * h + HD * f
    o, p0 = divmod(r0, 128)
    len1 = min(HD, 128 - p0)
    return o, p0, len1, HD - len1


def _build(qk_bias: bool, p_bias: bool):
    """Build + compile the single-core Bass program (shared SPMD across cores)."""
    key = (qk_bias, p_bias)
    if key in _BUILD_CACHE:
        return _BUILD_CACHE[key]

    nc = bacc.Bacc("TRN2", target_bir_lowering=False, debug=False)

    xT_d = nc.dram_tensor("xT", [C, BPC * N], BF16, kind="ExternalInput")
    wqk_d = nc.dram_tensor("wqk", [C, 2 * NH * HD], BF16, kind="ExternalInput")
    wv_d = nc.dram_tensor("wv", [C, VW], BF16, kind="ExternalInput")
    wp_d = nc.dram_tensor("wp", [C, C], BF16, kind="ExternalInput")
    bvaug_d = nc.dram_tensor("bvaug", [1, VW], BF16, kind="ExternalInput")
    ones_d = nc.dram_tensor("ones", [1, 128], BF16, kind="ExternalInput")
    vones_d = nc.dram_tensor("vones", [128, TOKC, NH], BF16, kind="ExternalInput")
    if qk_bias:
        # per-dense-row bias, column c = bias vector for chunk c's 128 rows
        bqkd_d = nc.dram_tensor("bqkd", [128, NCH], F32, kind="ExternalInput")
    if p_bias:
        bp_d = nc.dram_tensor("bp", [1, C], BF16, kind="ExternalInput")
    y_d = nc.dram_tensor("y", [BPC, N, C], F32, kind="ExternalOutput")

    xT_re = xT_d.rearrange("(o p) t -> p o t", p=128)
    wqk_re = wqk_d.rearrange("(o p) f -> p o f", p=128)
    wv_re = wv_d.rearrange("(o p) f -> p o f", p=128)
    wp_re = wp_d.rearrange("(o p) f -> p o f", p=128)

    EXP = mybir.ActivationFunctionType.Exp
    COPY = mybir.ActivationFunctionType.Copy
    MULT = mybir.AluOpType.mult

    with tile.TileContext(nc) as tc:
        with (
            tc.tile_pool(name="wpool", bufs=1) as wpool,
            tc.tile_pool(name="wqkh_pool", bufs=3) as wqkh_pool,
            tc.tile_pool(name="xpool", bufs=2) as xpool,
            tc.tile_pool(name="qktd_pool", bufs=5) as qktd_pool,
            tc.tile_pool(name="qkt_pool", bufs=4) as qkt_pool,
            tc.tile_pool(name="vpool", bufs=2) as vpool,
            tc.tile_pool(name="pt_pool", bufs=3) as pt_pool,
            tc.tile_pool(name="rbn_pool", bufs=2) as rbn_pool,
            tc.tile_pool(name="onat_pool", bufs=2) as onat_pool,
            tc.tile_pool(name="attn_pool", bufs=2) as attn_pool,
            tc.tile_pool(name="out_pool", bufs=6) as out_pool,
            tc.tile_pool(name="spsum", bufs=2, space="PSUM") as spsum,
            tc.tile_pool(name="opsum_pool", bufs=2, space="PSUM") as opsum_pool,
            tc.tile_pool(name="mpsum", bufs=2, space="PSUM") as mpsum,
        ):
            # --- resident weights/constants (x-t0 and wv-lo first: they
            # gate the very first V-projection group) ---
            wv_sb = wpool.tile([128, CC, VW], BF16, tag="wv")
            wp_sb = wpool.tile([128, CC, C], BF16, tag="wp")
            bvaug_sb = wpool.tile([1, VW], BF16, tag="bvaug")
            ones_sb = wpool.tile([1, 128], BF16, tag="ones")
            xTb0 = xpool.tile([128, CC, N], BF16, tag="xTb", name="xTb0")
            nc.sync.dma_start(xTb0[:, :, 0:128], xT_re[:, :, 0:128])
            nc.sync.dma_start(wv_sb[:, :, 0:512], wv_re[:, :, 0:512])
            nc.sync.dma_start(xTb0[:, :, 128:512], xT_re[:, :, 128:512])
            nc.sync.dma_start(wv_sb[:, :, 512:VW], wv_re[:, :, 512:VW])
            nc.sync.dma_start(xTb0[:, :, 512:N], xT_re[:, :, 512:N])
            nc.sync.dma_start(bvaug_sb[:], bvaug_d[:])
            nc.sync.dma_start(ones_sb[:], ones_d[:])
            if qk_bias:
                bqkd_sb = wpool.tile([128, NCH], F32, tag="bqkd")
                nc.sync.dma_start(bqkd_sb[:], bqkd_d[:])
            if p_bias:
                bp_sb = wpool.tile([1, C], BF16, tag="bp")
                nc.sync.dma_start(bp_sb[:], bp_d[:])

            def stage_x(b):
                """Issue batch b's x^T staging DMAs (2 big descriptors)."""
                xTb = xpool.tile([128, CC, N], BF16, tag="xTb", name="xTb")
                for lo, hi in ((0, 512), (512, N)):
                    nc.sync.dma_start(
                        xTb[:, :, lo:hi],
                        xT_re[:, :, b * N + lo : b * N + hi],
                    )
                return xTb

            def gen_v(b, xTb, v_sb):
                """V projection steps (16 PSUM groups) for batch b."""
                v_bias = bool(qk_bias)  # b_qkv nonzero => v bias nonzero path
                for lo, hi in ((0, 512), (512, VW)):
                    for t in range(TOKC):
                        vps = mpsum.tile([128, 512], F32, tag="mpsum", name="vps")
                        w = hi - lo
                        for cc in range(CC):
                            nc.tensor.matmul(
                                vps[:, :w],
                                xTb[:, cc, t * 128 : (t + 1) * 128],
                                wv_sb[:, cc, lo:hi],
                                start=(cc == 0),
                                stop=(cc == CC - 1 and not v_bias),
                            )
                            if cc == 2:
                                yield CC * w // 2
                        if v_bias:
                            # bias + per-head ones-columns via rank-1 update
                            nc.tensor.matmul(
                                vps[:, :w],
                                ones_sb[:, :],
                                bvaug_sb[:, lo:hi],
                                start=False,
                                stop=True,
                            )
                        nc.vector.tensor_copy(v_sb[:, t, lo:hi], vps[:, :w])
                        yield CC * w - CC * w // 2
                if not v_bias:
                    # fill each head's ones-column with a single strided DMA
                    nc.sync.dma_start(
                        v_sb.rearrange("p t (h a) -> p t h a", a=HDA)[:, :, :, HD],
                        vones_d[:],
                    )

            def make_qkchunks(b, xTb, prefetch_first=False):
                """Dense Q^T/K^T projection for batch b.

                Returns (gen, src): gen yields after each emission step
                (half-chunk projection group or realign-DMA bundle; 3 steps
                per chunk, 36 total) and src(h, f) -> AP of head-tile (h, f)
                as [96, N] rows at partition 0 (either a realigned tile or a
                direct qktd view).
                """
                qktd = {}  # chunk -> tile [128, N]
                qkt_tiles = {}  # h -> tile [96, 2, N]
                srcs = {}
                wqkh_groups = {}  # g -> tile [128, CC, 384] (chunks 3g..3g+2)

                # (h, f) tiles completing at chunk c (i.e. last row in c)
                finish = {c: [] for c in range(NCH)}
                for h in range(NH):
                    for f in range(2):
                        o, p0, len1, len2 = _qk_tile_geom(h, f)
                        finish[o + (1 if len2 else 0)].append((h, f))

                def load_group(g):
                    wqkh = wqkh_pool.tile(
                        [128, CC, 384], BF16, tag="wqkh", name="wqkh"
                    )
                    nc.sync.dma_start(
                        wqkh[:], wqk_re[:, :, 384 * g : 384 * (g + 1)]
                    )
                    wqkh_groups[g] = wqkh

                if prefetch_first:
                    load_group(0)

                def steps():
                    if not prefetch_first:
                        load_group(0)
                    for c in range(NCH):
                        if c % 3 == 0 and (c // 3) + 1 < NCH // 3:
                            load_group(c // 3 + 1)  # prefetch next group
                        wqkh = wqkh_groups[c // 3]
                        ws = slice((c % 3) * 128, (c % 3 + 1) * 128)
                        qktd_c = qktd_pool.tile([128, N], BF16, tag="qktd", name="qktd")
                        qktd[c] = qktd_c
                        for tq in range(QH):
                            qps = mpsum.tile([128, 512], F32, tag="mpsum", name="qps")
                            for cc in range(CC):
                                nc.tensor.matmul(
                                    qps[:, :],
                                    wqkh[:, cc, ws],
                                    xTb[:, cc, tq * 512 : (tq + 1) * 512],
                                    start=(cc == 0),
                                    stop=(cc == CC - 1),
                                )
                                if cc == 2:
                                    yield DENSE_CYC // 2
                            dst = qktd_c[:, tq * 512 : (tq + 1) * 512]
                            if qk_bias:
                                nc.scalar.activation(
                                    dst, qps[:, :], COPY, bias=bqkd_sb[:, c : c + 1]
                                )
                            else:
                                nc.vector.tensor_copy(dst, qps[:, :])
                            yield DENSE_CYC // 2
                        # realign head tiles finishing with this chunk
                        for h, f in finish[c]:
                            o, p0, len1, len2 = _qk_tile_geom(h, f)
                            if p0 == 0:
                                srcs[(h, f)] = qktd[o][0:HD, :]
                                continue
                            if h not in qkt_tiles:
                                qkt_tiles[h] = qkt_pool.tile(
                                    [HD, 2, N], BF16, tag="qkt", name="qkt"
                                )
                            qt = qkt_tiles[h]
                            nc.sync.dma_start(
                                qt[0:len1, f, :], qktd[o][p0 : p0 + len1, :]
                            )
                            if len2:
                                nc.sync.dma_start(
                                    qt[len1:HD, f, :], qktd[o + 1][0:len2, :]
                                )
                            srcs[(h, f)] = qt[:, f, :]
                        yield 1

                return steps(), lambda h, f: srcs[(h, f)]

            # dense-projection PE-cycles that must be complete before head
            # h's scores: 2 tq steps per chunk, through the chunk holding the
            # last row of tile (h, f=1)
            def dense_need(h):
                return CHUNK_CYC * ((2 * HD * h + HD + HD - 1) // 128 + 1)

            class Driver:
                """Drains a chain of filler generators into the attention
                loop's PE-idle windows, paced fractionally by PE cycles."""

                def __init__(self, gens, total):
                    self.gens = list(gens)
                    self.total = total
                    self.driven = 0

                def drive_to(self, target):
                    target = min(target, self.total)
                    while self.driven < target and self.gens:
                        v = next(self.gens[0], None)
                        if v is None:
                            self.gens.pop(0)
                            continue
                        self.driven += v

                def finish(self):
                    # fully exhaust every generator: trailing emission after
                    # the last yield (y DMAs, vones) must still run
                    while self.gens:
                        v = next(self.gens[0], None)
                        if v is None:
                            self.gens.pop(0)
                            continue
                        self.driven += v

            def emit_heads(b, v_sb, driver, src, pre_driven, tr_eng):
                """Attention for all heads; writes normalized O into oNat
                ([q, (h, hd)] per q-tile) and returns the transposed attnT.

                driver's filler chain starts with this batch's remaining
                dense-projection steps (dense_need deadlines are enforced
                relative to pre_driven)."""
                oNat = onat_pool.tile([128, TOKC, C], BF16, tag="oNat", name="oNat")
                for h in range(NH):
                    driver.drive_to(dense_need(h) - pre_driven)
                    srcQ = src(h, 0)
                    srcK = src(h, 1)
                    ops = opsum_pool.tile(
                        [128, TOKC, 128], F32, tag="opsum", name="ops"
                    )
                    def emit_av(kc, pt):
                        for qt in range(TOKC):
                            qh, qi = divmod(qt, 4)
                            # PSUM start=True clears has_written for the WHOLE
                            # bank, so only the first q-tile region per bank
                            # may use it; the others rely on flags=0
                            # overwrite-where-unwritten for their first write.
                            nc.tensor.matmul(
                                ops[:, qt, 0:HDA],
                                pt[:, qh, qi * 128 : (qi + 1) * 128],
                                v_sb[:, kc, HDA * h : HDA * h + HDA],
                                start=(kc == 0 and qt % 4 == 0),
                                stop=(kc == KC - 1),
                                skip_group_check=True,
                            )

                    prev_pt = None
                    for kc in range(KC):
                        pt = pt_pool.tile([128, QH, 512], BF16, tag="pt", name="pt")
                        for qh in range(QH):
                            st = spsum.tile([128, 512], F32, tag="spsum", name="st")
                            nc.tensor.matmul(
                                st[:, :],
                                srcK[:, kc * 128 : (kc + 1) * 128],
                                srcQ[:, qh * 512 : (qh + 1) * 512],
                                start=True,
                                stop=True,
                            )
                            nc.scalar.activation(
                                pt[:, qh, :], st[:, :], EXP, scale=SCALE
                            )
                        # keep the PE fed through the exp latency: fractional
                        # pacing of the filler chain across all 64 kc-steps,
                        # and run the PREVIOUS kc's AV (its exp is long done)
                        idx = 8 * h + kc
                        driver.drive_to(-(-driver.total * (idx + 1) // 64))
                        if prev_pt is not None:
                            emit_av(kc - 1, prev_pt)
                        prev_pt = pt
                    emit_av(KC - 1, prev_pt)
                    # normalize: per-partition reciprocal row-sum, broadcast
                    # along the free dim by tensor_scalar
                    rbn = rbn_pool.tile([128, TOKC], F32, tag="rbn", name="rbn")
                    nc.vector.reciprocal(rbn[:, :], ops[:, :, HD])
                    for qt in range(TOKC):
                        nc.vector.tensor_scalar(
                            oNat[:, qt, HD * h : HD * (h + 1)],
                            ops[:, qt, 0:HD],
                            rbn[:, qt : qt + 1],
                            None,
                            MULT,
                        )
                # repack to [C, q] layout: one XBAR transpose per q-tile
                attnT = attn_pool.tile(
                    [128, QC, CC, 128], BF16, tag="attnT", name="attnT"
                )
                for qt in range(TOKC):
                    tr_eng.dma_start_transpose(
                        attnT[:, qt, :, :], oNat[:, qt, :]
                    )
                return attnT

            def gen_proj(b, attnT, tail=False, qcs=None):
                """Output projection steps (2 PSUM groups per qc)."""
                for qc in qcs if qcs is not None else range(QC):
                    out_sb = out_pool.tile([128, C], F32, tag="out", name="out_sb")
                    for n in range(2):
                        pps = mpsum.tile([128, 512], F32, tag="mpsum", name="pps")
                        ns = slice(n * 384, (n + 1) * 384)
                        for cc in range(CC):
                            nc.tensor.matmul(
                                pps[:, :384],
                                attnT[:, qc, cc, :],
                                wp_sb[:, cc, ns],
                                start=(cc == 0),
                                stop=(cc == CC - 1 and not p_bias),
                            )
                            if cc == 2:
                                yield PROJ_CYC // 2
                        if p_bias:
                            nc.tensor.matmul(
                                pps[:, :384],
                                ones_sb[:, :],
                                bp_sb[:, ns],
                                start=False,
                                stop=True,
                            )
                        if n == 0 and tail:
                            nc.scalar.activation(out_sb[:, ns], pps[:, :384], COPY)
                        else:
                            nc.vector.tensor_copy(out_sb[:, ns], pps[:, :384])
                        yield PROJ_CYC - PROJ_CYC // 2
                    nc.sync.dma_start(y_d[b, qc * 128 : (qc + 1) * 128, :], out_sb[:])

            def take(gen, n):
                for _ in range(n):
                    v = next(gen, None)
                    if v is None:
                        return
                    yield v

            # Emission plan: V0 runs unoverlapped (nothing precedes it); the
            # attention kc-loops of batch 0 absorb [dense-rest(0), V(1),
            # dense-pre(1)] as PE filler; batch 1's absorb [dense-rest(1),
            # proj(0)]; proj(1) is the tail.
            v0 = vpool.tile([128, TOKC, VW], BF16, tag="v", name="v0")
            for _ in gen_v(0, xTb0, v0):
                pass
            g0, src0 = make_qkchunks(0, xTb0)
            # chunks 0-1 (+realign) must precede head 0's scores
            Driver([g0], 2 * CHUNK_CYC).drive_to(2 * CHUNK_CYC)
            xTb1 = stage_x(1)
            nc.sync.dma_start(wp_sb[:], wp_re[:])
            v1 = vpool.tile([128, TOKC, VW], BF16, tag="v", name="v1")
            g1, src1 = make_qkchunks(1, xTb1, prefetch_first=True)
            V_CYC = 8 * CC * 512 + 8 * CC * 264
            drv0 = Driver(
                [g0, gen_v(1, xTb1, v1), take(g1, 10)],
                10 * CHUNK_CYC + V_CYC + 2 * CHUNK_CYC,
            )
            at0 = emit_heads(
                0, v0, drv0, src0, pre_driven=2 * CHUNK_CYC, tr_eng=nc.sync
            )
            drv0.finish()
            drv1 = Driver(
                [g1, gen_proj(0, at0, qcs=(0, 1, 2, 3))],
                10 * CHUNK_CYC + 8 * PROJ_CYC,
            )
            at1 = emit_heads(
                1, v1, drv1, src1, pre_driven=2 * CHUNK_CYC, tr_eng=nc.sync
            )
            drv1.finish()
            # bridge batch 1's tail with the reserved proj(0) half
            for _ in gen_proj(0, at0, qcs=(4, 5, 6, 7)):
                pass
            for _ in gen_proj(1, at1, tail=True):
                pass

    nc.compile()
    _BUILD_CACHE[key] = nc
    return nc


def _prep_shared(w_qkv, b_qkv, w_proj, b_proj):
    """Host-side weight rearrangement shared by all cores."""
    w_qkv = np.ascontiguousarray(w_qkv, dtype=np.float32)
    w_proj = np.ascontiguousarray(w_proj, dtype=np.float32)
    b_qkv = np.asarray(b_qkv, dtype=np.float32)
    b_proj = np.asarray(b_proj, dtype=np.float32)

    # wqk: [C, 2*NH*HD] with column 2*HD*h + HD*f + j = w_qkv row C*f + HD*h + j
    wqk = w_qkv[: 2 * C].reshape(2, NH, HD, C)  # [f, h, j, c]
    wqk_arr = np.ascontiguousarray(
        np.transpose(wqk, (3, 1, 0, 2)).reshape(C, 2 * NH * HD).astype(NPBF16)
    )

    # wv: [C, NH*(HD+1)] with a zero ones-column slot per head
    wv = w_qkv[2 * C :].reshape(NH, HD, C)  # [h, j, c]
    wv_aug = np.zeros((C, NH, HDA), dtype=NPBF16)
    wv_aug[:, :, :HD] = np.transpose(wv, (2, 0, 1)).astype(NPBF16)
    wv_aug = np.ascontiguousarray(wv_aug.reshape(C, VW))

    # wp: plain transpose [c_in, c_out]
    wp_t = np.ascontiguousarray(w_proj.T.astype(NPBF16))

    # bvaug: v-bias interleaved with 1.0 at each head's ones-column
    bvaug = np.zeros((1, NH, HDA), dtype=np.float32)
    bvaug[0, :, :HD] = b_qkv[2 * C :].reshape(NH, HD)
    bvaug[0, :, HD] = 1.0
    bvaug = bvaug.reshape(1, VW).astype(NPBF16)

    ones = np.ones((1, 128), dtype=NPBF16)
    vones = np.ones((128, TOKC, NH), dtype=NPBF16)

    qk_bias = bool(np.any(b_qkv[: 2 * C] != 0.0))
    p_bias = bool(np.any(b_proj != 0.0))
    extra = {}
    if qk_bias:
        # dense-row order (h, f, j), reshaped so column c = chunk c's rows
        bqk = b_qkv[: 2 * C].reshape(2, NH, HD)  # [f, h, j]
        dense = np.transpose(bqk, (1, 0, 2)).reshape(2 * NH * HD)
        extra["bqkd"] = np.ascontiguousarray(dense.reshape(NCH, 128).T)
    if p_bias:
        extra["bp"] = np.ascontiguousarray(b_proj.reshape(1, C).astype(NPBF16))

    return wqk_arr, wv_aug, wp_t, bvaug, ones, vones, qk_bias, p_bias, extra


def kernel(x, w_qkv, b_qkv, w_proj, b_proj, H=32, W=32):
    x = np.asarray(x, dtype=np.float32)
    assert x.shape == (B, N, C), x.shape
    assert int(H) * int(W) == N

    wqk_arr, wv_aug, wp_t, bvaug, ones, vones, qk_bias, p_bias, extra = _prep_shared(
        w_qkv, b_qkv, w_proj, b_proj
    )
    nc = _build(qk_bias, p_bias)

    in_maps = []
    for c in range(NCORES):
        xc = x[BPC * c : BPC * (c + 1)].reshape(BPC * N, C)
        xT = np.ascontiguousarray(xc.T.astype(NPBF16))  # [C, BPC*N]
        m = {
            "xT": xT,
            "wqk": wqk_arr,
            "wv": wv_aug,
            "wp": wp_t,
            "bvaug": bvaug,
            "ones": ones,
            "vones": vones,
        }
        m.update(extra)
        in_maps.append(m)

    trace = os.environ.get("KERNEL_TRACE") == "1"
    res = run_bass_kernel_spmd(
        nc, in_maps, core_ids=list(range(NCORES)), trace=trace
    )
    if trace:
        kernel.last_results = res
        print("exec_time_ns:", res.exec_time_ns, "mean:", res.mean_exec_time_ns)
        if res.instructions_and_trace:
            print("trace:", res.instructions_and_trace[1])

    out = np.empty((B, N, C), dtype=np.float32)
    for c in range(NCORES):
        out[BPC * c : BPC * (c + 1)] = res.results[c]["y"]
    return out


if __name__ == "__main__":
    rng = np.random.default_rng(0)
    x = rng.standard_normal((B, N, C), dtype=np.float32)
    w_qkv = rng.standard_normal((3 * C, C), dtype=np.float32) / np.sqrt(C)
    b_qkv = np.zeros(3 * C, np.float32)
    w_proj = rng.standard_normal((C, C), dtype=np.float32) / np.sqrt(C)
    b_proj = np.zeros(C, np.float32)
    y = kernel(x, w_qkv, b_qkv, w_proj, b_proj)
    print("out", y.shape, y.dtype, float(np.abs(y).mean()))


# revision 30
# speedup vs baseline: 1.2157x; 1.0029x over previous
"""Trainium2 Bass kernel for multi-head global attention (the
"DeformableAttention" module whose relative-position-bias path is inactive).

Reference computation (per batch b):
    qkv = x @ w_qkv.T + b_qkv            # [N, 3C]
    q, k, v = split/reshape to [nh, N, hd]
    attn = softmax((q @ k.T) * hd**-0.5)
    out  = (attn @ v) merged heads       # [N, C]
    y    = out @ w_proj.T + b_proj

Sharding: data-parallel over batch B=16 across 8 NeuronCores (2 batches/core).
No collectives.

Device-side design (per core, per batch), all SBUF operands bf16 so every
matmul streams at 1 cycle/row and 128-column stationaries get fast weight
load:
  * x is staged pre-transposed (xT, [C, tokens]).
  * Q^T/K^T are produced by a DENSE projection: 12 chunks of M=128 rows in
    (head, q|k, j) order -- no M=96 underutilization. Head tiles that start
    mid-chunk are realigned to partition 0 by small SBUF->SBUF DMAs; the 4
    chunk-aligned tiles are read in place.
  * V in natural [N, nh*(hd+1)] layout with an interleaved ones-column per
    head.
  * Scores are computed transposed (S^T[k, q] blocks), softmax's exp runs on
    ScalarE with the 1/sqrt(hd) scale fused.
  * AV runs QUERY-NATURAL: out[q, hd] tiles with M=128 (full PE height),
    F=97 bf16; the ones-column row-sum lands on the same partition as its
    queries, so the normalize is a per-partition reciprocal + tensor_scalar
    (no cross-partition broadcast, no DRAM bounce).
  * The normalized per-q-tile [128, C] block (heads side by side) is turned
    into the projection's [C, q] layout by one XBAR DMA-transpose per q-tile.
  * Output projection contracts 6 dense 128-chunks, producing [token, C]
    tiles for direct DMA out.
"""

import os
import sys

sys.path.insert(0, "/opt/trn_rl_repo")

# The Bass->PJRT execution path needs jax to discover the axon-tunneled
# NeuronCores; a stray JAX_PLATFORMS=cpu (e.g. set for a jax reference run)
# would hide them. Only effective if jax hasn't been imported yet.
if "jax" not in sys.modules and "axon" not in os.environ.get("JAX_PLATFORMS", "axon"):
    os.environ.pop("JAX_PLATFORMS", None)

import ml_dtypes
import numpy as np

import concourse.bass as bass
import concourse.mybir as mybir
import concourse.tile as tile
from concourse import bacc
from concourse.bass_utils import run_bass_kernel_spmd

# Problem constants (hardcoded per the task contract).
B, N, C = 16, 1024, 768
NH, HD = 8, 96
NCORES = 8
BPC = B // NCORES  # batches per core = 2
CC = C // 128  # contraction chunks of 128 = 6
KC = N // 128  # key chunks per batch = 8
QH = N // 512  # query halves = 2
TOKC = N // 128  # token chunks = 8
QC = N // 128  # query chunks for output projection = 8
HDA = HD + 1  # head dim + ones column = 97
VW = NH * HDA  # augmented V width = 776
NCH = 2 * NH * HD // 128  # dense Q/K projection chunks = 12
SCALE = float(HD) ** -0.5

F32 = mybir.dt.float32
BF16 = mybir.dt.bfloat16
DENSE_CYC = 6 * 512  # PE cycles of one dense-projection tq step
CHUNK_CYC = 2 * DENSE_CYC + 1  # tq steps + the (1-cycle) realign step
PROJ_CYC = 6 * 384  # PE cycles of one output-projection group
NPBF16 = ml_dtypes.bfloat16

_BUILD_CACHE = {}


def _qk_tile_geom(h, f):
    """Dense-row geometry of head-tile (h, f): rows r0..r0+95 of the
    (h, f, j) row space land in chunk o at partitions p0.., possibly
    spilling len2 rows into chunk o+1."""
    r0 = 2 * HD * h + HD * f
    o, p0 = divmod(r0, 128)
    len1 = min(HD, 128 - p0)
    return o, p0, len1, HD - len1


def _build(qk_bias: bool, p_bias: bool):
    """Build + compile the single-core Bass program (shared SPMD across cores)."""
    key = (qk_bias, p_bias)
    if key in _BUILD_CACHE:
        return _BUILD_CACHE[key]

    nc = bacc.Bacc("TRN2", target_bir_lowering=False, debug=False)

    xT_d = nc.dram_tensor("xT", [C, BPC * N], BF16, kind="ExternalInput")
    wqk_d = nc.dram_tensor("wqk", [C, 2 * NH * HD], BF16, kind="ExternalInput")
    wv_d = nc.dram_tensor("wv", [C, VW], BF16, kind="ExternalInput")
    wp_d = nc.dram_tensor("wp", [C, C], BF16, kind="ExternalInput")
    bvaug_d = nc.dram_tensor("bvaug", [1, VW], BF16, kind="ExternalInput")
    ones_d = nc.dram_tensor("ones", [1, 128], BF16, kind="ExternalInput")
    vones_d = nc.dram_tensor("vones", [128, TOKC, NH], BF16, kind="ExternalInput")
    if qk_bias:
        # per-dense-row bias, column c = bias vector for chunk c's 128 rows
        bqkd_d = nc.dram_tensor("bqkd", [128, NCH], F32, kind="ExternalInput")
    if p_bias:
        bp_d = nc.dram_tensor("bp", [1, C], BF16, kind="ExternalInput")
    y_d = nc.dram_tensor("y", [BPC, N, C], F32, kind="ExternalOutput")

    xT_re = xT_d.rearrange("(o p) t -> p o t", p=128)
    wqk_re = wqk_d.rearrange("(o p) f -> p o f", p=128)
    wv_re = wv_d.rearrange("(o p) f -> p o f", p=128)
    wp_re = wp_d.rearrange("(o p) f -> p o f", p=128)

    EXP = mybir.ActivationFunctionType.Exp
    COPY = mybir.ActivationFunctionType.Copy
    MULT = mybir.AluOpType.mult

    with tile.TileContext(nc) as tc:
        with (
            tc.tile_pool(name="wpool", bufs=1) as wpool,
            tc.tile_pool(name="wqkh_pool", bufs=3) as wqkh_pool,
            tc.tile_pool(name="xpool", bufs=2) as xpool,
            tc.tile_pool(name="qktd_pool", bufs=5) as qktd_pool,
            tc.tile_pool(name="qkt_pool", bufs=4) as qkt_pool,
            tc.tile_pool(name="vpool", bufs=2) as vpool,
            tc.tile_pool(name="pt_pool", bufs=3) as pt_pool,
            tc.tile_pool(name="rbn_pool", bufs=2) as rbn_pool,
            tc.tile_pool(name="onat_pool", bufs=2) as onat_pool,
            tc.tile_pool(name="attn_pool", bufs=2) as attn_pool,
            tc.tile_pool(name="out_pool", bufs=6) as out_pool,
            tc.tile_pool(name="spsum", bufs=2, space="PSUM") as spsum,
            tc.tile_pool(name="opsum_pool", bufs=2, space="PSUM") as opsum_pool,
            tc.tile_pool(name="mpsum", bufs=2, space="PSUM") as mpsum,
        ):
            # --- resident weights/constants (x-t0 and wv-lo first: they
            # gate the very first V-projection group) ---
            wv_sb = wpool.tile([128, CC, VW], BF16, tag="wv")
            wp_sb = wpool.tile([128, CC, C], BF16, tag="wp")
            bvaug_sb = wpool.tile([1, VW], BF16, tag="bvaug")
            ones_sb = wpool.tile([1, 128], BF16, tag="ones")
            xTb0 = xpool.tile([128, CC, N], BF16, tag="xTb", name="xTb0")
            nc.sync.dma_start(xTb0[:, :, 0:128], xT_re[:, :, 0:128])
            nc.sync.dma_start(wv_sb[:, :, 512:VW], wv_re[:, :, 512:VW])
            nc.sync.dma_start(xTb0[:, :, 128:512], xT_re[:, :, 128:512])
            nc.sync.dma_start(wv_sb[:, :, 0:512], wv_re[:, :, 0:512])
            nc.sync.dma_start(xTb0[:, :, 512:N], xT_re[:, :, 512:N])
            nc.sync.dma_start(bvaug_sb[:], bvaug_d[:])
            nc.sync.dma_start(ones_sb[:], ones_d[:])
            if qk_bias:
                bqkd_sb = wpool.tile([128, NCH], F32, tag="bqkd")
                nc.sync.dma_start(bqkd_sb[:], bqkd_d[:])
            if p_bias:
                bp_sb = wpool.tile([1, C], BF16, tag="bp")
                nc.sync.dma_start(bp_sb[:], bp_d[:])

            def stage_x(b):
                """Issue batch b's x^T staging DMAs (2 big descriptors)."""
                xTb = xpool.tile([128, CC, N], BF16, tag="xTb", name="xTb")
                for lo, hi in ((0, 512), (512, N)):
                    nc.sync.dma_start(
                        xTb[:, :, lo:hi],
                        xT_re[:, :, b * N + lo : b * N + hi],
                    )
                return xTb

            def gen_v(b, xTb, v_sb):
                """V projection steps (16 PSUM groups) for batch b."""
                v_bias = bool(qk_bias)  # b_qkv nonzero => v bias nonzero path
                for lo, hi in ((512, VW), (0, 512)):
                    for t in range(TOKC):
                        vps = mpsum.tile([128, 512], F32, tag="mpsum", name="vps")
                        w = hi - lo
                        for cc in range(CC):
                            nc.tensor.matmul(
                                vps[:, :w],
                                xTb[:, cc, t * 128 : (t + 1) * 128],
                                wv_sb[:, cc, lo:hi],
                                start=(cc == 0),
                                stop=(cc == CC - 1 and not v_bias),
                            )
                            if cc == 2:
                                yield CC * w // 2
                        if v_bias:
                            # bias + per-head ones-columns via rank-1 update
                            nc.tensor.matmul(
                                vps[:, :w],
                                ones_sb[:, :],
                                bvaug_sb[:, lo:hi],
                                start=False,
                                stop=True,
                            )
                        nc.vector.tensor_copy(v_sb[:, t, lo:hi], vps[:, :w])
                        yield CC * w - CC * w // 2
                if not v_bias:
                    # fill each head's ones-column with a single strided DMA
                    nc.sync.dma_start(
                        v_sb.rearrange("p t (h a) -> p t h a", a=HDA)[:, :, :, HD],
                        vones_d[:],
                    )

            def make_qkchunks(b, xTb, prefetch_first=False):
                """Dense Q^T/K^T projection for batch b.

                Returns (gen, src): gen yields after each emission step
                (half-chunk projection group or realign-DMA bundle; 3 steps
                per chunk, 36 total) and src(h, f) -> AP of head-tile (h, f)
                as [96, N] rows at partition 0 (either a realigned tile or a
                direct qktd view).
                """
                qktd = {}  # chunk -> tile [128, N]
                qkt_tiles = {}  # h -> tile [96, 2, N]
                srcs = {}
                wqkh_groups = {}  # g -> tile [128, CC, 384] (chunks 3g..3g+2)

                # (h, f) tiles completing at chunk c (i.e. last row in c)
                finish = {c: [] for c in range(NCH)}
                for h in range(NH):
                    for f in range(2):
                        o, p0, len1, len2 = _qk_tile_geom(h, f)
                        finish[o + (1 if len2 else 0)].append((h, f))

                def load_group(g):
                    wqkh = wqkh_pool.tile(
                        [128, CC, 384], BF16, tag="wqkh", name="wqkh"
                    )
                    nc.sync.dma_start(
                        wqkh[:], wqk_re[:, :, 384 * g : 384 * (g + 1)]
                    )
                    wqkh_groups[g] = wqkh

                if prefetch_first:
                    load_group(0)

                def steps():
                    if not prefetch_first:
                        load_group(0)
                    for c in range(NCH):
                        if c % 3 == 0 and (c // 3) + 1 < NCH // 3:
                            load_group(c // 3 + 1)  # prefetch next group
                        wqkh = wqkh_groups[c // 3]
                        ws = slice((c % 3) * 128, (c % 3 + 1) * 128)
                        qktd_c = qktd_pool.tile([128, N], BF16, tag="qktd", name="qktd")
                        qktd[c] = qktd_c
                        for tq in range(QH):
                            qps = mpsum.tile([128, 512], F32, tag="mpsum", name="qps")
                            for cc in range(CC):
                                nc.tensor.matmul(
                                    qps[:, :],
                                    wqkh[:, cc, ws],
                                    xTb[:, cc, tq * 512 : (tq + 1) * 512],
                                    start=(cc == 0),
                                    stop=(cc == CC - 1),
                                )
                                if cc == 2:
                                    yield DENSE_CYC // 2
                            dst = qktd_c[:, tq * 512 : (tq + 1) * 512]
                            if qk_bias:
                                nc.scalar.activation(
                                    dst, qps[:, :], COPY, bias=bqkd_sb[:, c : c + 1]
                                )
                            else:
                                nc.vector.tensor_copy(dst, qps[:, :])
                            yield DENSE_CYC // 2
                        # realign head tiles finishing with this chunk
                        for h, f in finish[c]:
                            o, p0, len1, len2 = _qk_tile_geom(h, f)
                            if p0 == 0:
                                srcs[(h, f)] = qktd[o][0:HD, :]
                                continue
                            if h not in qkt_tiles:
                                qkt_tiles[h] = qkt_pool.tile(
                                    [HD, 2, N], BF16, tag="qkt", name="qkt"
                                )
                            qt = qkt_tiles[h]
                            nc.sync.dma_start(
                                qt[0:len1, f, :], qktd[o][p0 : p0 + len1, :]
                            )
                            if len2:
                                nc.sync.dma_start(
                                    qt[len1:HD, f, :], qktd[o + 1][0:len2, :]
                                )
                            srcs[(h, f)] = qt[:, f, :]
                        yield 1

                return steps(), lambda h, f: srcs[(h, f)]

            # dense-projection PE-cycles that must be complete before head
            # h's scores: 2 tq steps per chunk, through the chunk holding the
            # last row of tile (h, f=1)
            def dense_need(h):
                return CHUNK_CYC * ((2 * HD * h + HD + HD - 1) // 128 + 1)

            class Driver:
                """Drains a chain of filler generators into the attention
                loop's PE-idle windows, paced fractionally by PE cycles."""

                def __init__(self, gens, total):
                    self.gens = list(gens)
                    self.total = total
                    self.driven = 0

                def drive_to(self, target):
                    target = min(target, self.total)
                    while self.driven < target and self.gens:
                        v = next(self.gens[0], None)
                        if v is None:
                            self.gens.pop(0)
                            continue
                        self.driven += v

                def finish(self):
                    # fully exhaust every generator: trailing emission after
                    # the last yield (y DMAs, vones) must still run
                    while self.gens:
                        v = next(self.gens[0], None)
                        if v is None:
                            self.gens.pop(0)
                            continue
                        self.driven += v

            def emit_heads(b, v_sb, driver, src, pre_driven, tr_eng):
                """Attention for all heads; writes normalized O into oNat
                ([q, (h, hd)] per q-tile) and returns the transposed attnT.

                driver's filler chain starts with this batch's remaining
                dense-projection steps (dense_need deadlines are enforced
                relative to pre_driven)."""
                oNat = onat_pool.tile([128, TOKC, C], BF16, tag="oNat", name="oNat")
                for h in range(NH):
                    driver.drive_to(dense_need(h) - pre_driven)
                    srcQ = src(h, 0)
                    srcK = src(h, 1)
                    ops = opsum_pool.tile(
                        [128, TOKC, 128], F32, tag="opsum", name="ops"
                    )
                    def emit_av(kc, pt):
                        for qt in range(TOKC):
                            qh, qi = divmod(qt, 4)
                            # PSUM start=True clears has_written for the WHOLE
                            # bank, so only the first q-tile region per bank
                            # may use it; the others rely on flags=0
                            # overwrite-where-unwritten for their first write.
                            nc.tensor.matmul(
                                ops[:, qt, 0:HDA],
                                pt[:, qh, qi * 128 : (qi + 1) * 128],
                                v_sb[:, kc, HDA * h : HDA * h + HDA],
                                start=(kc == 0 and qt % 4 == 0),
                                stop=(kc == KC - 1),
                                skip_group_check=True,
                            )

                    prev_pt = None
                    for kc in range(KC):
                        pt = pt_pool.tile([128, QH, 512], BF16, tag="pt", name="pt")
                        for qh in range(QH):
                            st = spsum.tile([128, 512], F32, tag="spsum", name="st")
                            nc.tensor.matmul(
                                st[:, :],
                                srcK[:, kc * 128 : (kc + 1) * 128],
                                srcQ[:, qh * 512 : (qh + 1) * 512],
                                start=True,
                                stop=True,
                            )
                            nc.scalar.activation(
                                pt[:, qh, :], st[:, :], EXP, scale=SCALE
                            )
                        # keep the PE fed through the exp latency: fractional
                        # pacing of the filler chain across all 64 kc-steps,
                        # and run the PREVIOUS kc's AV (its exp is long done)
                        idx = 8 * h + kc
                        driver.drive_to(-(-driver.total * (idx + 1) // 64))
                        if prev_pt is not None:
                            emit_av(kc - 1, prev_pt)
                        prev_pt = pt
                    emit_av(KC - 1, prev_pt)
                    # normalize: per-partition reciprocal row-sum, broadcast
                    # along the free dim by tensor_scalar
                    rbn = rbn_pool.tile([128, TOKC], F32, tag="rbn", name="rbn")
                    nc.vector.reciprocal(rbn[:, :], ops[:, :, HD])
                    for qt in range(TOKC):
                        nc.vector.tensor_scalar(
                            oNat[:, qt, HD * h : HD * (h + 1)],
                            ops[:, qt, 0:HD],
                            rbn[:, qt : qt + 1],
                            None,
                            MULT,
                        )
                # repack to [C, q] layout: one XBAR transpose per q-tile
                attnT = attn_pool.tile(
                    [128, QC, CC, 128], BF16, tag="attnT", name="attnT"
                )
                for qt in range(TOKC):
                    tr_eng.dma_start_transpose(
                        attnT[:, qt, :, :], oNat[:, qt, :]
                    )
                return attnT

            def gen_proj(b, attnT, tail=False, qcs=None):
                """Output projection steps (2 PSUM groups per qc)."""
                for qc in qcs if qcs is not None else range(QC):
                    out_sb = out_pool.tile([128, C], F32, tag="out", name="out_sb")
                    for n in range(2):
                        pps = mpsum.tile([128, 512], F32, tag="mpsum", name="pps")
                        ns = slice(n * 384, (n + 1) * 384)
                        for cc in range(CC):
                            nc.tensor.matmul(
                                pps[:, :384],
                                attnT[:, qc, cc, :],
                                wp_sb[:, cc, ns],
                                start=(cc == 0),
                                stop=(cc == CC - 1 and not p_bias),
                            )
                            if cc == 2:
                                yield PROJ_CYC // 2
                        if p_bias:
                            nc.tensor.matmul(
                                pps[:, :384],
                                ones_sb[:, :],
                                bp_sb[:, ns],
                                start=False,
                                stop=True,
                            )
                        if n == 0 and tail:
                            nc.scalar.activation(out_sb[:, ns], pps[:, :384], COPY)
                        else:
                            nc.vector.tensor_copy(out_sb[:, ns], pps[:, :384])
                        if tail:
                            nc.sync.dma_start(
                                y_d[b, qc * 128 : (qc + 1) * 128, ns],
                                out_sb[:, ns],
                            )
                        yield PROJ_CYC - PROJ_CYC // 2
                    if not tail:
                        nc.sync.dma_start(
                            y_d[b, qc * 128 : (qc + 1) * 128, :], out_sb[:]
                        )

            def take(gen, n):
                for _ in range(n):
                    v = next(gen, None)
                    if v is None:
                        return
                    yield v

            # Emission plan: V0 runs unoverlapped (nothing precedes it); the
            # attention kc-loops of batch 0 absorb [dense-rest(0), V(1),
            # dense-pre(1)] as PE filler; batch 1's absorb [dense-rest(1),
            # proj(0)]; proj(1) is the tail.
            v0 = vpool.tile([128, TOKC, VW], BF16, tag="v", name="v0")
            for _ in gen_v(0, xTb0, v0):
                pass
            g0, src0 = make_qkchunks(0, xTb0)
            # chunks 0-1 (+realign) must precede head 0's scores
            Driver([g0], 2 * CHUNK_CYC).drive_to(2 * CHUNK_CYC)
            xTb1 = stage_x(1)
            nc.sync.dma_start(wp_sb[:], wp_re[:])
            v1 = vpool.tile([128, TOKC, VW], BF16, tag="v", name="v1")
            g1, src1 = make_qkchunks(1, xTb1, prefetch_first=True)
            V_CYC = 8 * CC * 512 + 8 * CC * 264
            drv0 = Driver(
                [g0, gen_v(1, xTb1, v1), take(g1, 10)],
                10 * CHUNK_CYC + V_CYC + 2 * CHUNK_CYC,
            )
            at0 = emit_heads(
                0, v0, drv0, src0, pre_driven=2 * CHUNK_CYC, tr_eng=nc.sync
            )
            drv0.finish()
            drv1 = Driver(
                [g1, gen_proj(0, at0, qcs=(0, 1, 2, 3))],
                10 * CHUNK_CYC + 8 * PROJ_CYC,
            )
            at1 = emit_heads(
                1, v1, drv1, src1, pre_driven=2 * CHUNK_CYC, tr_eng=nc.sync
            )
            drv1.finish()
            # bridge batch 1's normalize/transpose tail with the reserved
            # proj(0) half
            for _ in gen_proj(0, at0, qcs=(4, 5, 6, 7)):
                pass
            for _ in gen_proj(1, at1, tail=True):
                pass

    nc.compile()
    _BUILD_CACHE[key] = nc
    return nc


def _prep_shared(w_qkv, b_qkv, w_proj, b_proj):
    """Host-side weight rearrangement shared by all cores."""
    w_qkv = np.ascontiguousarray(w_qkv, dtype=np.float32)
    w_proj = np.ascontiguousarray(w_proj, dtype=np.float32)
    b_qkv = np.asarray(b_qkv, dtype=np.float32)
    b_proj = np.asarray(b_proj, dtype=np.float32)

    # wqk: [C, 2*NH*HD] with column 2*HD*h + HD*f + j = w_qkv row C*f + HD*h + j
    wqk = w_qkv[: 2 * C].reshape(2, NH, HD, C)  # [f, h, j, c]
    wqk_arr = np.ascontiguousarray(
        np.transpose(wqk, (3, 1, 0, 2)).reshape(C, 2 * NH * HD).astype(NPBF16)
    )

    # wv: [C, NH*(HD+1)] with a zero ones-column slot per head
    wv = w_qkv[2 * C :].reshape(NH, HD, C)  # [h, j, c]
    wv_aug = np.zeros((C, NH, HDA), dtype=NPBF16)
    wv_aug[:, :, :HD] = np.transpose(wv, (2, 0, 1)).astype(NPBF16)
    wv_aug = np.ascontiguousarray(wv_aug.reshape(C, VW))

    # wp: plain transpose [c_in, c_out]
    wp_t = np.ascontiguousarray(w_proj.T.astype(NPBF16))

    # bvaug: v-bias interleaved with 1.0 at each head's ones-column
    bvaug = np.zeros((1, NH, HDA), dtype=np.float32)
    bvaug[0, :, :HD] = b_qkv[2 * C :].reshape(NH, HD)
    bvaug[0, :, HD] = 1.0
    bvaug = bvaug.reshape(1, VW).astype(NPBF16)

    ones = np.ones((1, 128), dtype=NPBF16)
    vones = np.ones((128, TOKC, NH), dtype=NPBF16)

    qk_bias = bool(np.any(b_qkv[: 2 * C] != 0.0))
    p_bias = bool(np.any(b_proj != 0.0))
    extra = {}
    if qk_bias:
        # dense-row order (h, f, j), reshaped so column c = chunk c's rows
        bqk = b_qkv[: 2 * C].reshape(2, NH, HD)  # [f, h, j]
        dense = np.transpose(bqk, (1, 0, 2)).reshape(2 * NH * HD)
        extra["bqkd"] = np.ascontiguousarray(dense.reshape(NCH, 128).T)
    if p_bias:
        extra["bp"] = np.ascontiguousarray(b_proj.reshape(1, C).astype(NPBF16))

    return wqk_arr, wv_aug, wp_t, bvaug, ones, vones, qk_bias, p_bias, extra


def kernel(x, w_qkv, b_qkv, w_proj, b_proj, H=32, W=32):
    x = np.asarray(x, dtype=np.float32)
    assert x.shape == (B, N, C), x.shape
    assert int(H) * int(W) == N

    wqk_arr, wv_aug, wp_t, bvaug, ones, vones, qk_bias, p_bias, extra = _prep_shared(
        w_qkv, b_qkv, w_proj, b_proj
    )
    nc = _build(qk_bias, p_bias)

    in_maps = []
    for c in range(NCORES):
        xc = x[BPC * c : BPC * (c + 1)].reshape(BPC * N, C)
        xT = np.ascontiguousarray(xc.T.astype(NPBF16))  # [C, BPC*N]
        m = {
            "xT": xT,
            "wqk": wqk_arr,
            "wv": wv_aug,
            "wp": wp_t,
            "bvaug": bvaug,
            "ones": ones,
            "vones": vones,
        }
        m.update(extra)
        in_maps.append(m)

    trace = os.environ.get("KERNEL_TRACE") == "1"
    res = run_bass_kernel_spmd(
        nc, in_maps, core_ids=list(range(NCORES)), trace=trace
    )
    if trace:
        kernel.last_results = res
        print("exec_time_ns:", res.exec_time_ns, "mean:", res.mean_exec_time_ns)
        if res.instructions_and_trace:
            print("trace:", res.instructions_and_trace[1])

    out = np.empty((B, N, C), dtype=np.float32)
    for c in range(NCORES):
        out[BPC * c : BPC * (c + 1)] = res.results[c]["y"]
    return out


if __name__ == "__main__":
    rng = np.random.default_rng(0)
    x = rng.standard_normal((B, N, C), dtype=np.float32)
    w_qkv = rng.standard_normal((3 * C, C), dtype=np.float32) / np.sqrt(C)
    b_qkv = np.zeros(3 * C, np.float32)
    w_proj = rng.standard_normal((C, C), dtype=np.float32) / np.sqrt(C)
    b_proj = np.zeros(C, np.float32)
    y = kernel(x, w_qkv, b_qkv, w_proj, b_proj)
    print("out", y.shape, y.dtype, float(np.abs(y).mean()))
